# revision 1
# baseline (speedup 1.0000x reference)
"""MultiHeadLatentAttention (MLA) Trainium2 Bass kernel.

Problem: B=2, S=2048, D=2048, H=16 heads, d_nope=128, d_rope=64, d_head=128,
q_latent=768, kv_latent=512. Causal attention, rmsnorm'd latents, half-dim RoPE.

Sharding (8 cores): core c handles batch b=c//4 and head group g=c%4 (4 heads).
The small latent down-projections are replicated within each batch group;
W_uq/W_qr/W_uk/W_kr/W_uv are column-sharded by head; W_o row-sharded; the
4 partial outputs per batch are summed on the host.

Device dataflow (everything in "transposed" layout, features on partitions,
sequence on the free dim, so every matmul uses weights as-stored for lhsT and
all moving operands have free dim 512):
  P0: q_latT/kv_latT = W_d*^T @ x^T, rmsnorm via ones-matmul sumsq +
      exp(-0.5*ln(ms)) + gpsimd partition_broadcast; latents round-trip DRAM.
  P1: kT (nope + rope pairs) and v (natural layout) up-projections.
  P2: per 512-wide q-block: q up-proj on demand, scores^T = k^T(tile)^T q^T
      with additive causal mask applied via an identity matmul of a
      precomputed mask; exp batched over [128,1024] two-bank PSUM tiles on
      ACT; softmax denominator accumulated on the DVE and finished with a
      gpsimd partition_all_reduce (broadcast sum), 1/den = exp(-ln(den));
      PV matmuls (staggered one exp-pair behind the scores matmuls) give
      out^T; then y^T += W_o^T out^T, deferred one q-block for overlap.
      Projection chains alternate between two PSUM pools for 4-deep
      chain pipelining within the 8-bank budget.

All matmuls run as float32r (FP22 multiply, fp32 accumulate, 1 cycle/row with
512-wide moving operands) — measured end-to-end relative error vs the fp32
reference is ~4e-4.

MLA_ALLGATHER=1 switches to an S-sharded down-projection with a device
AllGather of the latents within each 4-core batch group (saves ~110us of
replicated down-projection matmuls per core, but the cost model prices the
10.5MB gather at ~250us, so it is off by default).
"""
import math
import os
from contextlib import ExitStack

import numpy as np

import concourse.bass as bass
import concourse.bass_isa as bass_isa
import concourse.bacc as bacc
import concourse.mybir as mybir
import concourse.tile as tile
from concourse.bass_utils import run_bass_kernel_spmd

F32 = mybir.dt.float32
F32R = mybir.dt.float32r
AF = mybir.ActivationFunctionType

B, S_FULL, D = 2, 2048, 2048
H, DN, DR, DH = 16, 128, 64, 128
QL, KVL = 768, 512
EPS = 1e-6
SCALE = 1.0 / math.sqrt(DH)
MASK_NEG = -1e6
NCORES = 8
NKT = D // 128          # 16 contraction tiles over D
NLQ = QL // 128         # 6
NLKV = KVL // 128       # 4
NDT = D // 128          # 16 output D tiles


def _rope_apply(nc, pool, ps, c4s, s4s, out_ap):
    """Half-dim rope on a pair tile [128, 512] (h_even x1|x2 | h_odd x1|x2).

    out = ps * c4 + shuf(ps) * s4,  shuf swaps the 32-blocks within each 64.
    ps is PSUM; out_ap is SBUF.
    """
    shuf = pool.tile([128, 512], F32, tag="rope_shuf")
    nc.vector.tensor_copy(shuf[0:32, :], ps[32:64, :])
    nc.vector.tensor_copy(shuf[32:64, :], ps[0:32, :])
    nc.vector.tensor_copy(shuf[64:96, :], ps[96:128, :])
    nc.vector.tensor_copy(shuf[96:128, :], ps[64:96, :])
    t1 = pool.tile([128, 512], F32, tag="rope_t1")
    nc.vector.tensor_mul(t1[:], ps[:], c4s)
    nc.vector.tensor_mul(out_ap, shuf[:], s4s)
    nc.vector.tensor_add(out_ap, out_ap, t1[:])


PHASE_MARKS = {}


def build_nc(S=S_FULL, allgather=None):
    assert S % 512 == 0
    n_sb = S // 512
    n_st = S // 128
    if allgather is None:
        allgather = bool(int(os.environ.get("MLA_ALLGATHER", "0")))
    PHASE_MARKS.clear()

    nc = bacc.Bacc("TRN2", target_bir_lowering=False, debug=False,
                   num_devices=NCORES)

    x_cols = 512 if allgather else S
    xT_d = nc.dram_tensor("xT", [D, x_cols], F32R, kind="ExternalInput")
    wdq_d = nc.dram_tensor("W_dq", [D, QL], F32R, kind="ExternalInput")
    wdkv_d = nc.dram_tensor("W_dkv", [D, KVL], F32R, kind="ExternalInput")
    wuq_d = nc.dram_tensor("Wuq", [QL, 512], F32R, kind="ExternalInput")
    wqr_d = nc.dram_tensor("Wqr", [QL, 256], F32R, kind="ExternalInput")
    wuk_d = nc.dram_tensor("Wuk", [KVL, 512], F32R, kind="ExternalInput")
    wkr_d = nc.dram_tensor("Wkr", [KVL, 256], F32R, kind="ExternalInput")
    wuv_d = nc.dram_tensor("Wuv", [KVL, 512], F32R, kind="ExternalInput")
    wo_d = nc.dram_tensor("Wo", [512, D], F32R, kind="ExternalInput")
    c4_d = nc.dram_tensor("c4", [128, S], F32, kind="ExternalInput")
    s4_d = nc.dram_tensor("s4", [128, S], F32, kind="ExternalInput")
    mask_d = nc.dram_tensor("mask_big", [128, 896], F32R, kind="ExternalInput")
    id_d = nc.dram_tensor("ident", [128, 128], F32R, kind="ExternalInput")
    ones_d = nc.dram_tensor("ones_col", [128, 1], F32R, kind="ExternalInput")
    yT_d = nc.dram_tensor("yT", [D, S], F32, kind="ExternalOutput")

    def col3(dram_ap, p=128):
        # [R, C] dram slice -> [128, R//128, C] tiled AP
        return dram_ap.rearrange("(t p) c -> p t c", p=p)

    with tile.TileContext(nc) as tc:
        with (
            tc.tile_pool(name="const", bufs=1) as constp,
            tc.tile_pool(name="ps_mm", bufs=2, space="PSUM") as ps_mm,
            tc.tile_pool(name="ps_o", bufs=2, space="PSUM") as ps_op,
            tc.tile_pool(name="dram", bufs=1, space="DRAM") as dramp,
        ):
            if allgather:
                lat_in = dramp.tile([QL + KVL, 512], F32R)
                lat_out = dramp.tile([n_sb * (QL + KVL), 512], F32R)

                def qlat_src(sb):
                    return lat_out[sb * 1280:sb * 1280 + QL, :]

                def kvlat_src(sb):
                    return lat_out[sb * 1280 + QL:(sb + 1) * 1280, :]
            else:
                qlat_ds = [
                    dramp.tile([QL, 512], F32R, tag=f"qlat{i}", name=f"qlat{i}")
                    for i in range(n_sb)
                ]
                kvlat_ds = [
                    dramp.tile([KVL, 512], F32R, tag=f"kvlat{i}",
                               name=f"kvlat{i}")
                    for i in range(n_sb)
                ]

                def qlat_src(sb):
                    return qlat_ds[sb][:]

                def kvlat_src(sb):
                    return kvlat_ds[sb][:]
            def alt_ps(i):
                if i % 2 == 0:
                    return ps_mm.tile([128, 512], F32, tag="mm", name="ps")
                return ps_op.tile([128, 512], F32, tag="pv", name="ps")

            mask_t = constp.tile([128, 896], F32R)
            id_t = constp.tile([128, 128], F32R)
            ones_t = constp.tile([128, 1], F32R)
            eps_t = constp.tile([1, 1], F32)
            # ---------------- P0: down-projections + rmsnorm ----------------
            PHASE_MARKS["P0"] = nc.next_id()
            p1_stack = ExitStack()
            p1lat = p1_stack.enter_context(tc.tile_pool(name="p1lat", bufs=2))
            kvl_tiles = {}
            with (
                tc.tile_pool(name="p0w", bufs=1) as p0w,
                tc.tile_pool(name="p0x", bufs=4) as p0x,
                tc.tile_pool(name="p0latq", bufs=2) as p0latq,
                tc.tile_pool(name="p0latkv", bufs=1) as p0latkv,
                tc.tile_pool(name="p0tmp", bufs=1) as p0tmp,
                tc.tile_pool(name="p0rsb", bufs=2) as p0rsb,
                tc.tile_pool(name="ps_den", bufs=1, space="PSUM") as ps_denp,
                tc.tile_pool(name="ps_p0", bufs=2, space="PSUM") as ps_p0,
            ):
                def alt3_ps(i):
                    if i % 3 == 2:
                        return ps_p0.tile([128, 512], F32, tag="p0", name="ps")
                    return alt_ps(i % 3)
                wdq_t = p0w.tile([128, NKT, QL], F32R)
                wdkv_t = p0w.tile([128, NKT, KVL], F32R)

                def load_wdq(lt):
                    nc.sync.dma_start(
                        wdq_t[:, :, 128 * lt:128 * (lt + 1)],
                        col3(wdq_d[:, 128 * lt:128 * (lt + 1)]),
                    )

                for g4 in range(4):
                    nc.sync.dma_start(
                        wdq_t[:, 4 * g4:4 * (g4 + 1), 0:128],
                        col3(wdq_d[:, 0:128])[:, 4 * g4:4 * (g4 + 1), :],
                    )
                n_local = 1 if allgather else n_sb
                for sb in range(n_local):
                    cs = slice(512 * sb, 512 * (sb + 1))
                    xh = [
                        p0x.tile([128, 8, 512], F32R, tag="xsb", name=f"xsb{i}")
                        for i in range(2)
                    ]
                    for kt in range(NKT):
                        nc.sync.dma_start(
                            xh[kt // 8][:, kt % 8, :],
                            col3(xT_d[:, cs])[:, kt, :],
                        )
                    if sb == 0:
                        nc.sync.dma_start(ones_t[:], ones_d[:])
                        nc.vector.memset(eps_t[:], EPS)
                        for lt in range(1, NLQ):
                            load_wdq(lt)
                        for lt in range(NLKV):
                            nc.sync.dma_start(
                                wdkv_t[:, :, 128 * lt:128 * (lt + 1)],
                                col3(wdkv_d[:, 128 * lt:128 * (lt + 1)]),
                            )
                    for latname, w_t, nl in (("q", wdq_t, NLQ), ("kv", wdkv_t, NLKV)):
                        raw = (p0latq if latname == "q" else p0latkv).tile(
                            [128, nl, 512], F32R, tag=f"raw{latname}",
                            name=f"raw{latname}")
                        ps_ss = ps_denp.tile([1, 512], F32, tag="den")
                        for lt in range(nl):
                            ps = alt3_ps(lt)
                            for kt in range(NKT):
                                nc.tensor.matmul(
                                    ps[:],
                                    w_t[:, kt, 128 * lt:128 * (lt + 1)],
                                    xh[kt // 8][:, kt % 8, :],
                                    start=(kt == 0), stop=(kt == NKT - 1),
                                )
                            nc.scalar.copy(raw[:, lt, :], ps[:])
                            sq = p0tmp.tile([128, 512], F32R, tag="sq")
                            nc.vector.tensor_mul(sq[:], raw[:, lt, :], raw[:, lt, :])
                            nc.tensor.matmul(
                                ps_ss[:], ones_t[:], sq[:],
                                start=(lt == 0), stop=(lt == nl - 1),
                            )
                        lrow = p0tmp.tile([1, 512], F32, tag="lrow")
                        nc.scalar.activation(
                            lrow[:], ps_ss[:], AF.Ln, scale=1.0 / (128 * nl),
                            bias=eps_t[:],
                        )
                        rrow = p0tmp.tile([1, 512], F32R, tag="rrow")
                        nc.scalar.activation(rrow[:], lrow[:], AF.Exp, scale=-0.5)
                        rsb = p0rsb.tile([128, 512], F32R, tag="rsb")
                        nc.gpsimd.partition_broadcast(rsb[:], rrow[:])
                        for lt in range(nl):
                            nc.vector.tensor_mul(raw[:, lt, :], raw[:, lt, :], rsb[:])
                        if allgather:
                            lat_ap = (lat_in[0:QL, :] if latname == "q"
                                      else lat_in[QL:QL + KVL, :])
                        else:
                            lat_ap = (qlat_ds if latname == "q"
                                      else kvlat_ds)[sb][:]
                        nc.sync.dma_start(col3(lat_ap), raw[:])
                    if sb == 0 and not allgather:
                        kvl0 = p1lat.tile([128, NLKV, 512], F32R, tag="kvl",
                                          name="kvl0")
                        nc.sync.dma_start(kvl0[:], col3(kvlat_src(0)))
                        kvl_tiles[0] = kvl0

            nc.sync.dma_start(mask_t[:], mask_d[:])
            nc.sync.dma_start(id_t[:], id_d[:])
            if allgather:
                nc.gpsimd.collective_compute(
                    "AllGather",
                    mybir.AluOpType.bypass,
                    replica_groups=[[0, 1, 2, 3], [4, 5, 6, 7]],
                    ins=[lat_in[:]],
                    outs=[lat_out[:]],
                )

            # ---------------- P1: k/v up-projections ----------------
            PHASE_MARKS["P1"] = nc.next_id()
            persist_stack = ExitStack()
            persist = persist_stack.enter_context(
                tc.tile_pool(name="persist", bufs=1)
            )
            kTn_t = persist.tile([128, 4, S], F32R)    # nope k^T per head
            kTr_t = persist.tile([128, 2, S], F32R)    # rope k^T per pair
            v_t = persist.tile([128, n_st, 512], F32R)  # v natural
            p2w_stack = ExitStack()
            p2w = p2w_stack.enter_context(tc.tile_pool(name="p2w", bufs=1))
            p2lat = p2w_stack.enter_context(tc.tile_pool(name="p2lat", bufs=1))
            wuq_t = p2w.tile([128, NLQ, 512], F32R)
            wqr_t = p2w.tile([128, NLQ, 256], F32R)
            qlat0 = p2lat.tile([128, NLQ, 512], F32R, tag="qlat", name="qlat0")
            with (
                tc.tile_pool(name="p1w", bufs=1) as p1w,
                tc.tile_pool(name="p1tmp", bufs=2) as p1tmp,
            ):
                wuk_t = p1w.tile([128, NLKV, 512], F32R)
                wkr_t = p1w.tile([128, NLKV, 256], F32R)
                wuv_t = p1w.tile([128, NLKV, 512], F32R)
                nc.sync.dma_start(wuk_t[:], col3(wuk_d[:]))
                for sb in range(n_sb):
                    cs = slice(512 * sb, 512 * (sb + 1))
                    if sb in kvl_tiles:
                        kvl = kvl_tiles[sb]
                    else:
                        kvl = p1lat.tile([128, NLKV, 512], F32R, tag="kvl",
                                         name=f"kvl{sb}")
                        nc.sync.dma_start(kvl[:], col3(kvlat_src(sb)))
                    if sb == 0:
                        nc.sync.dma_start(wkr_t[:], col3(wkr_d[:]))
                        nc.sync.dma_start(wuv_t[:], col3(wuv_d[:]))
                    c4s = p1tmp.tile([128, 512], F32, tag="c4")
                    s4s = p1tmp.tile([128, 512], F32, tag="s4")
                    nc.sync.dma_start(c4s[:], c4_d[:, cs])
                    nc.sync.dma_start(s4s[:], s4_d[:, cs])
                    for h in range(4):
                        ps = alt_ps(h)
                        for kl in range(NLKV):
                            nc.tensor.matmul(
                                ps[:], wuk_t[:, kl, 128 * h:128 * (h + 1)],
                                kvl[:, kl, :],
                                start=(kl == 0), stop=(kl == NLKV - 1),
                            )
                        nc.scalar.copy(kTn_t[:, h, cs], ps[:])
                    for pr in range(2):
                        ps = alt_ps(pr)
                        for kl in range(NLKV):
                            nc.tensor.matmul(
                                ps[:], wkr_t[:, kl, 128 * pr:128 * (pr + 1)],
                                kvl[:, kl, :],
                                start=(kl == 0), stop=(kl == NLKV - 1),
                            )
                        _rope_apply(nc, p1tmp, ps, c4s[:], s4s[:], kTr_t[:, pr, cs])
                    for stl in range(4):
                        st = 4 * sb + stl
                        ps = alt_ps(stl)
                        for kl in range(NLKV):
                            nc.tensor.matmul(
                                ps[:],
                                kvl[:, kl, 128 * stl:128 * (stl + 1)],
                                wuv_t[:, kl, :],
                                start=(kl == 0), stop=(kl == NLKV - 1),
                            )
                        nc.scalar.copy(v_t[:, st, :], ps[:])
                    if sb == min(1, n_sb - 1):
                        nc.sync.dma_start(wuq_t[:], col3(wuq_d[:]))
                        nc.sync.dma_start(wqr_t[:], col3(wqr_d[:]))
                        nc.sync.dma_start(qlat0[:], col3(qlat_src(0)))

            # ---------------- P2: attention + W_o ----------------
            PHASE_MARKS["P2"] = nc.next_id()
            with (
                tc.tile_pool(name="p2wo", bufs=3) as p2wo,
                tc.tile_pool(name="p2q", bufs=4) as p2q,
                tc.tile_pool(name="p2qr", bufs=2) as p2qr,
                tc.tile_pool(name="p2exp", bufs=2) as p2exp,
                tc.tile_pool(name="ps_s", bufs=2, space="PSUM") as ps_sp,
                tc.tile_pool(name="p2acc", bufs=2) as p2acc,
                tc.tile_pool(name="p2acc1", bufs=1) as p2acc1,
                tc.tile_pool(name="p2out", bufs=9) as p2out,
                tc.tile_pool(name="p2tmp", bufs=2) as p2tmp,
                tc.tile_pool(name="p2y", bufs=2) as p2y,
            ):
                def emit_wo(out_tiles, cs):
                    for dt in range(NDT):
                        woc = p2wo.tile([128, 4, 128], F32R, tag="wo", name="woc")
                        nc.sync.dma_start(
                            woc[:], col3(wo_d[:, 128 * dt:128 * (dt + 1)])
                        )
                        ps_y = ps_mm.tile([128, 512], F32, tag="mm", name="ps")
                        for h in range(4):
                            nc.tensor.matmul(
                                ps_y[:], woc[:, h, :],
                                out_tiles[h][:],
                                start=(h == 0), stop=(h == 3),
                            )
                        ystage = p2y.tile([128, 512], F32, tag="y")
                        nc.vector.tensor_copy(ystage[:], ps_y[:])
                        nc.sync.dma_start(
                            yT_d[128 * dt:128 * (dt + 1), cs], ystage[:]
                        )

                prev_out = None
                prev_cs = None
                for qb in range(n_sb):
                    cs = slice(512 * qb, 512 * (qb + 1))
                    if qb == 0:
                        qlat = qlat0
                    else:
                        qlat = p2lat.tile([128, NLQ, 512], F32R, tag="qlat",
                                          name=f"qlat{qb}")
                        nc.sync.dma_start(qlat[:], col3(qlat_src(qb)))
                    c4s = p2tmp.tile([128, 512], F32, tag="c4")
                    s4s = p2tmp.tile([128, 512], F32, tag="s4")
                    nc.sync.dma_start(c4s[:], c4_d[:, cs])
                    nc.sync.dma_start(s4s[:], s4_d[:, cs])
                    qr_tiles = []
                    for pr in range(2):
                        ps = alt_ps(pr)
                        for ql in range(NLQ):
                            nc.tensor.matmul(
                                ps[:], wqr_t[:, ql, 128 * pr:128 * (pr + 1)],
                                qlat[:, ql, :],
                                start=(ql == 0), stop=(ql == NLQ - 1),
                            )
                        qr = p2qr.tile([128, 512], F32R, tag="qr")
                        _rope_apply(nc, p2tmp, ps, c4s[:], s4s[:], qr[:])
                        qr_tiles.append(qr)
                    qn_tiles = []
                    for h in range(4):
                        ps = alt_ps(h)
                        for ql in range(NLQ):
                            nc.tensor.matmul(
                                ps[:], wuq_t[:, ql, 128 * h:128 * (h + 1)],
                                qlat[:, ql, :],
                                start=(ql == 0), stop=(ql == NLQ - 1),
                            )
                        qn = p2q.tile([128, 512], F32R, tag="qn", name=f"qn{h}")
                        nc.vector.tensor_copy(qn[:], ps[:])
                        qn_tiles.append(qn)
                    out_tiles = []
                    for h in range(4):
                        qn = qn_tiles[h]
                        qr = qr_tiles[h // 2]
                        pb = 64 * (h % 2)
                        nkt = 4 * (qb + 1)
                        ps_o = ps_op.tile([128, 512], F32, tag="pv")
                        # softmax denominator: DVE accumulates exp half 0,
                        # GPSIMD half 1; exp batches two score tiles per ACT op.
                        dacc = p2acc.tile([128, 512], F32R, tag="dacc")

                        def emit_pv(exp_pair, pk, npair, ps_o=ps_o, h=h):
                            for j in (0, 1):
                                kt = 2 * pk + j
                                nc.tensor.matmul(
                                    ps_o[:],
                                    v_t[:, kt, 128 * h:128 * (h + 1)],
                                    exp_pair[:, 512 * j:512 * (j + 1)],
                                    start=(kt == 0), stop=(kt == 2 * npair - 1),
                                )

                        npair = nkt // 2
                        pend = []   # (exp pair tile, pk) one pair behind
                        for pk in range(npair):
                            ps_s = ps_sp.tile([128, 1024], F32, tag="scores")
                            for j in (0, 1):
                                kt = 2 * pk + j
                                ks = slice(128 * kt, 128 * (kt + 1))
                                delta = 128 * kt - 512 * qb
                                diag = delta >= 0
                                half = ps_s[:, 512 * j:512 * (j + 1)]
                                nc.tensor.matmul(
                                    half, kTn_t[:, h, ks], qn[:],
                                    start=True, stop=False,
                                )
                                nc.tensor.matmul(
                                    half,
                                    kTr_t[pb:pb + 64, h // 2, ks],
                                    qr[pb:pb + 64, :],
                                    start=False, stop=not diag,
                                )
                                if diag:
                                    nc.tensor.matmul(
                                        half, id_t[:],
                                        mask_t[:, 384 - delta:896 - delta],
                                        start=False, stop=True,
                                    )
                            exp_t = p2exp.tile([128, 1024], F32R, tag="exp")
                            nc.scalar.activation(
                                exp_t[:], ps_s[:], AF.Exp, scale=SCALE
                            )
                            if pk == 0:
                                nc.vector.tensor_copy(dacc[:], exp_t[:, 0:512])
                            else:
                                nc.vector.tensor_add(
                                    dacc[:], dacc[:], exp_t[:, 0:512])
                            nc.vector.tensor_add(
                                dacc[:], dacc[:], exp_t[:, 512:1024])
                            pend.append((exp_t, pk))
                            if len(pend) > 1:
                                emit_pv(*pend.pop(0), npair)
                        for e in pend:
                            emit_pv(*e, npair)
                        red = p2acc1.tile([128, 512], F32R, tag="dred")
                        nc.gpsimd.partition_all_reduce(
                            red[:], dacc[:], 128, bass_isa.ReduceOp.add
                        )
                        nc.scalar.activation(red[:], red[:], AF.Ln)
                        rsb = p2tmp.tile([128, 512], F32R, tag="rsb")
                        nc.scalar.activation(rsb[:], red[:], AF.Exp, scale=-1.0)
                        out_t = p2out.tile([128, 512], F32R, tag="outT")
                        nc.vector.tensor_mul(out_t[:], ps_o[:], rsb[:])
                        out_tiles.append(out_t)
                    if prev_out is not None:
                        emit_wo(prev_out, prev_cs)
                    prev_out, prev_cs = out_tiles, cs
                emit_wo(prev_out, prev_cs)
            p2w_stack.close()
            persist_stack.close()
            p1_stack.close()

    nc.compile()
    return nc


def host_prep(inputs, S=S_FULL):
    """Build the 8 per-core input maps from the full problem inputs."""
    x = np.ascontiguousarray(np.asarray(inputs["x"], np.float32))
    cosT = np.ascontiguousarray(np.asarray(inputs["rope_cos"], np.float32).T)
    sinT = np.ascontiguousarray(np.asarray(inputs["rope_sin"], np.float32).T)
    c4 = np.ascontiguousarray(np.concatenate([cosT, cosT, cosT, cosT], 0))
    s4 = np.ascontiguousarray(np.concatenate([-sinT, sinT, -sinT, sinT], 0))
    qw = np.asarray(inputs["q_norm_w"], np.float32)
    kvw = np.asarray(inputs["kv_norm_w"], np.float32)
    W_uq = np.asarray(inputs["W_uq"], np.float32) * qw[:, None]
    W_qr = np.asarray(inputs["W_qr"], np.float32) * qw[:, None]
    W_uk = np.asarray(inputs["W_uk"], np.float32) * kvw[:, None]
    W_kr = np.asarray(inputs["W_kr"], np.float32) * kvw[:, None]
    W_uv = np.asarray(inputs["W_uv"], np.float32) * kvw[:, None]
    W_o = np.asarray(inputs["W_o"], np.float32)
    W_dq = np.ascontiguousarray(np.asarray(inputs["W_dq"], np.float32))
    W_dkv = np.ascontiguousarray(np.asarray(inputs["W_dkv"], np.float32))

    cgrid = np.arange(896)[None, :] - 384
    igrid = np.arange(128)[:, None]
    mask_big = np.where(cgrid >= igrid, 0.0, MASK_NEG).astype(np.float32)
    ident = np.eye(128, dtype=np.float32)

    allgather = bool(int(os.environ.get("MLA_ALLGATHER", "0")))
    in_maps = []
    for c in range(NCORES):
        b, g = c // 4, c % 4
        hs = slice(4 * g * DN, 4 * (g + 1) * DN)
        hr = slice(4 * g * DR, 4 * (g + 1) * DR)
        xT_c = x[b].T[:, 512 * g:512 * (g + 1)] if allgather else x[b].T
        in_maps.append(dict(
            xT=np.ascontiguousarray(xT_c),
            W_dq=W_dq, W_dkv=W_dkv,
            Wuq=np.ascontiguousarray(W_uq[:, hs]),
            Wqr=np.ascontiguousarray(W_qr[:, hr]),
            Wuk=np.ascontiguousarray(W_uk[:, hs]),
            Wkr=np.ascontiguousarray(W_kr[:, hr]),
            Wuv=np.ascontiguousarray(W_uv[:, hs]),
            Wo=np.ascontiguousarray(W_o[512 * g:512 * (g + 1), :]),
            c4=c4, s4=s4, mask_big=mask_big, ident=ident,
            ones_col=np.ones((128, 1), np.float32),
        ))
    return in_maps


_NC_CACHE = {}


def kernel(**inputs) -> np.ndarray:
    S = np.asarray(inputs["x"]).shape[1]
    if S not in _NC_CACHE:
        _NC_CACHE[S] = build_nc(S)
    nc = _NC_CACHE[S]
    in_maps = host_prep(inputs, S)
    trace = bool(os.environ.get("MLA_TRACE"))
    res = run_bass_kernel_spmd(
        nc, in_maps, core_ids=list(range(NCORES)), trace=trace
    )
    if trace:
        print(f"HW exec time: {res.exec_time_ns} ns")
        print(f"trace: {res.instructions_and_trace[1] if res.instructions_and_trace else None}")
    y = np.empty((B, S, D), np.float32)
    for b in range(B):
        acc = res.results[4 * b]["yT"].astype(np.float32).copy()
        for g in range(1, 4):
            acc += res.results[4 * b + g]["yT"]
        y[b] = acc.T
    return y



# revision 9
# speedup vs baseline: 1.3225x; 1.3225x over previous
"""MultiHeadLatentAttention (MLA) Trainium2 Bass kernel, v2.

Problem: B=2, S=2048, D=2048, H=16 heads, d_nope=128, d_rope=64, d_head=128,
q_latent=768, kv_latent=512. Causal attention, rmsnorm'd latents, half-dim RoPE.

Sharding (8 cores): core c handles batch b=c//4 and head group g=c%4 (4 heads).
The small latent down-projections are replicated within each batch group;
W_uq/W_qr/W_uk/W_kr/W_uv are column-sharded by head; W_o row-sharded; the
4 partial outputs per batch are summed on the host.

v2 changes vs v1 (559us):
  * all matmul operands in bf16 (same PE rate as f32r in the cost model,
    half the DMA bytes and SBUF footprint; measured end-to-end rel err ~5e-3
    vs the 2e-2 gate).
  * latents stay resident in SBUF (no DRAM round trip).
  * ~40 large DMAs instead of 245 small ones (SP sequencer was 97% busy
    issuing DMAs in v1); W_o loaded once; x loaded in [128,8,512] halves.
  * rmsnorm reciprocal via a single AF.Rsqrt; softmax 1/den via a DVE
    divide. v1 used Ln+Exp pairs, which thrashed the activation-function
    table (49 LoadActFuncSet x 1.3us, most of it on the softmax critical
    path).
  * causal clipping: diagonal score/PV/mask matmuls only cover the
    unmasked column suffix (>=256 wide to stay at 1 cycle/row).
  * sumsq for rmsnorm via ACT Square + ones-matmul, emitted after the main
    chains so the PE never waits on it.
"""
import math
import os
from contextlib import ExitStack

import numpy as np
import ml_dtypes

import concourse.bass as bass
import concourse.bass_isa as bass_isa
import concourse.bacc as bacc
import concourse.mybir as mybir
import concourse.tile as tile
from concourse.bass_utils import run_bass_kernel_spmd

F32 = mybir.dt.float32
F32R = mybir.dt.float32r
BF16 = mybir.dt.bfloat16
AF = mybir.ActivationFunctionType
NPBF = ml_dtypes.bfloat16

B, S_FULL, D = 2, 2048, 2048
H, DN, DR, DH = 16, 128, 64, 128
QL, KVL = 768, 512
EPS = 1e-6
SCALE = 1.0 / math.sqrt(DH)
MASK_NEG = -1e6
NCORES = 8
NKT = D // 128          # 16 contraction tiles over D
NLQ = QL // 128         # 6
NLKV = KVL // 128       # 4
NDT = D // 128          # 16 output D tiles


def _rope_apply(nc, pool, ps, c4s, s4s, out_ap):
    """Half-dim rope on a pair tile [128, 512] (h_even x1|x2 | h_odd x1|x2).

    out = ps * c4 + shuf(ps) * s4,  shuf swaps the 32-blocks within each 64.
    ps is PSUM; out_ap is SBUF (bf16).
    """
    shuf = pool.tile([128, 512], F32, tag="rope_shuf")
    nc.vector.tensor_copy(shuf[0:32, :], ps[32:64, :])
    nc.vector.tensor_copy(shuf[32:64, :], ps[0:32, :])
    nc.vector.tensor_copy(shuf[64:96, :], ps[96:128, :])
    nc.vector.tensor_copy(shuf[96:128, :], ps[64:96, :])
    t1 = pool.tile([128, 512], F32, tag="rope_t1")
    nc.vector.tensor_mul(t1[:], ps[:], c4s)
    nc.vector.tensor_mul(shuf[:], shuf[:], s4s)
    nc.vector.tensor_add(out_ap, shuf[:], t1[:])


PHASE_MARKS = {}


def build_nc(S=S_FULL):
    assert S % 512 == 0
    n_sb = S // 512
    n_st = S // 128
    PHASE_MARKS.clear()

    nc = bacc.Bacc("TRN2", target_bir_lowering=False, debug=False,
                   num_devices=NCORES)

    xT_d = nc.dram_tensor("xT", [D, S], BF16, kind="ExternalInput")
    wdq_d = nc.dram_tensor("W_dq", [D, QL], BF16, kind="ExternalInput")
    wdkv_d = nc.dram_tensor("W_dkv", [D, KVL], BF16, kind="ExternalInput")
    wuq_d = nc.dram_tensor("Wuq", [QL, 512], BF16, kind="ExternalInput")
    wqr_d = nc.dram_tensor("Wqr", [QL, 256], BF16, kind="ExternalInput")
    wuk_d = nc.dram_tensor("Wuk", [KVL, 512], BF16, kind="ExternalInput")
    wkr_d = nc.dram_tensor("Wkr", [KVL, 256], BF16, kind="ExternalInput")
    wuv_d = nc.dram_tensor("Wuv", [KVL, 512], BF16, kind="ExternalInput")
    wo_d = nc.dram_tensor("Wo", [512, D], BF16, kind="ExternalInput")
    c4_d = nc.dram_tensor("c4", [128, S], F32, kind="ExternalInput")
    s4_d = nc.dram_tensor("s4", [128, S], F32, kind="ExternalInput")
    mask_d = nc.dram_tensor("mask_big", [128, 896], BF16, kind="ExternalInput")
    id_d = nc.dram_tensor("ident", [128, 128], BF16, kind="ExternalInput")
    ones_d = nc.dram_tensor("ones_col", [128, 1], BF16, kind="ExternalInput")
    yT_d = nc.dram_tensor("yT", [D, S], BF16, kind="ExternalOutput")

    def col3(dram_ap, p=128):
        # [R, C] dram slice -> [128, R//128, C] tiled AP
        return dram_ap.rearrange("(t p) c -> p t c", p=p)

    with tile.TileContext(nc) as tc:
        with (
            tc.tile_pool(name="const", bufs=1) as constp,
            tc.tile_pool(name="lat", bufs=1) as latp,
            tc.tile_pool(name="upw", bufs=1) as upw,
        ):
            mask_t = constp.tile([128, 896], BF16)
            id_t = constp.tile([128, 128], BF16)
            ones_t = constp.tile([128, 1], BF16)
            eps_t = constp.tile([1, 1], F32)
            c4_t = constp.tile([128, S], F32)
            s4_t = constp.tile([128, S], F32)
            qlat_t = latp.tile([128, NLQ, S], BF16)
            kvlat_t = latp.tile([128, NLKV, S], BF16)
            wuk_t = upw.tile([128, NLKV, 512], BF16)
            wkr_t = upw.tile([128, NLKV, 256], BF16)
            wuv_t = upw.tile([128, NLKV, 512], BF16)
            wuq_t = upw.tile([128, NLQ, 512], BF16)
            wqr_t = upw.tile([128, NLQ, 256], BF16)
            wo_t = upw.tile([128, 4, D], BF16)

            # ---------------- P0: down-projections + rmsnorm ----------------
            PHASE_MARKS["P0"] = nc.next_id()
            with (
                tc.tile_pool(name="p0w", bufs=1) as p0w,
                tc.tile_pool(name="p0x", bufs=4) as p0x,
                tc.tile_pool(name="p0raw", bufs=2) as p0raw,
                tc.tile_pool(name="p0sq", bufs=1) as p0sq,
                tc.tile_pool(name="p0r", bufs=2) as p0r,
                tc.tile_pool(name="ps_ch", bufs=3, space="PSUM") as ps_ch,
                tc.tile_pool(name="ps_den", bufs=2, space="PSUM") as ps_denp,
            ):
                wdq_t = p0w.tile([128, NKT, QL], BF16)
                wdkv_t = p0w.tile([128, NKT, KVL], BF16)

                def xh_tiles(sb):
                    return [
                        p0x.tile([128, 8, 512], BF16, tag="xsb",
                                 name=f"xsb{sb}_{i}")
                        for i in range(2)
                    ]

                def load_x(sb, xh):
                    cs = slice(512 * sb, 512 * (sb + 1))
                    for i in range(2):
                        nc.sync.dma_start(
                            xh[i][:],
                            col3(xT_d[:, cs])[:, 8 * i:8 * (i + 1), :],
                        )

                # startup loads, ordered for earliest PE start
                xh_cur = xh_tiles(0)
                nc.sync.dma_start(xh_cur[0][:], col3(xT_d[:, 0:512])[:, 0:8, :])
                nc.sync.dma_start(wdq_t[:, :, 0:384], col3(wdq_d[:, 0:384]))
                nc.sync.dma_start(xh_cur[1][:], col3(xT_d[:, 0:512])[:, 8:16, :])
                nc.sync.dma_start(wdq_t[:, :, 384:768], col3(wdq_d[:, 384:768]))
                nc.sync.dma_start(wdkv_t[:], col3(wdkv_d[:]))
                nc.sync.dma_start(c4_t[:], c4_d[:])
                nc.sync.dma_start(s4_t[:], s4_d[:])
                nc.sync.dma_start(mask_t[:], mask_d[:])
                nc.sync.dma_start(id_t[:], id_d[:])

                for sb in range(n_sb):
                    cs = slice(512 * sb, 512 * (sb + 1))
                    xh = xh_cur
                    if sb + 1 < n_sb:
                        xh_cur = xh_tiles(sb + 1)
                        load_x(sb + 1, xh_cur)
                    for latname, w_t, nl, lat_t in (
                        ("q", wdq_t, NLQ, qlat_t),
                        ("kv", wdkv_t, NLKV, kvlat_t),
                    ):
                        raw = p0raw.tile([128, nl, 512], BF16,
                                         tag=f"raw{latname}",
                                         name=f"raw{latname}")
                        # sumsq accumulated on the (otherwise idle) Pool
                        # engine so neither PE nor ACT carries it.
                        sqa = p0sq.tile([128, 512], F32, tag=f"sqa{latname}",
                                        name=f"sqa{latname}")
                        sqt = p0sq.tile([128, 512], F32, tag=f"sqt{latname}",
                                        name=f"sqt{latname}")
                        for lt in range(nl):
                            ps = ps_ch.tile([128, 512], F32, tag="ch")
                            for kt in range(NKT):
                                nc.tensor.matmul(
                                    ps[:],
                                    w_t[:, kt, 128 * lt:128 * (lt + 1)],
                                    xh[kt // 8][:, kt % 8, :],
                                    start=(kt == 0), stop=(kt == NKT - 1),
                                )
                            nc.scalar.copy(raw[:, lt, :], ps[:])
                            if lt == 0:
                                nc.gpsimd.tensor_mul(
                                    sqa[:], raw[:, 0, :], raw[:, 0, :])
                            else:
                                nc.gpsimd.tensor_mul(
                                    sqt[:], raw[:, lt, :], raw[:, lt, :])
                                nc.gpsimd.tensor_add(sqa[:], sqa[:], sqt[:])
                        sumb = p0r.tile([128, 512], F32, tag="sumb")
                        nc.gpsimd.partition_all_reduce(
                            sumb[:], sqa[:], 128, bass_isa.ReduceOp.add)
                        # r = 1/sqrt(mean+eps) without touching the Exp/Ln
                        # activation tables: Copy(scale,bias) -> DVE
                        # reciprocal -> Sqrt (Sqrt shares a table set with
                        # Copy, so P0 needs no table reloads).
                        ms_t = p0r.tile([128, 512], F32, tag="msrow")
                        nc.scalar.activation(
                            ms_t[:], sumb[:], AF.Copy,
                            scale=1.0 / (128 * nl), bias=float(EPS),
                        )
                        inv_t = p0r.tile([128, 512], F32, tag="invrow")
                        nc.vector.reciprocal(inv_t[:], ms_t[:])
                        rsb = p0r.tile([128, 512], BF16, tag="rsb")
                        nc.scalar.activation(rsb[:], inv_t[:], AF.Sqrt)
                        for lt in range(nl):
                            nc.vector.tensor_mul(
                                lat_t[:, lt, cs], raw[:, lt, :], rsb[:])
                    if sb == 0:
                        # up-projection weights: needed from P1 on; queue the
                        # transfers behind the first x prefetch.
                        nc.sync.dma_start(wuk_t[:], col3(wuk_d[:]))
                        nc.sync.dma_start(wkr_t[:], col3(wkr_d[:]))
                        nc.sync.dma_start(wuv_t[:], col3(wuv_d[:]))
                        nc.sync.dma_start(wuq_t[:], col3(wuq_d[:]))
                        nc.sync.dma_start(wqr_t[:], col3(wqr_d[:]))
                        nc.sync.dma_start(wo_t[:], col3(wo_d[:]))

            # ---------------- P1: k/v up-projections ----------------
            PHASE_MARKS["P1"] = nc.next_id()
            persist_stack = ExitStack()
            kvp = persist_stack.enter_context(tc.tile_pool(name="kvp", bufs=1))
            kTn_t = kvp.tile([128, 4, S], BF16)    # nope k^T per head
            kTr_t = kvp.tile([128, 2, S], BF16)    # rope k^T per pair
            v_t = kvp.tile([128, n_st, 512], BF16)  # v natural
            with (
                tc.tile_pool(name="ps_p1", bufs=3, space="PSUM") as ps_p1,
                tc.tile_pool(name="p1tmp", bufs=2) as p1tmp,
            ):
                for sb in range(n_sb):
                    cs = slice(512 * sb, 512 * (sb + 1))
                    for h in range(4):
                        ps = ps_p1.tile([128, 512], F32, tag="ch")
                        for kl in range(NLKV):
                            nc.tensor.matmul(
                                ps[:], wuk_t[:, kl, 128 * h:128 * (h + 1)],
                                kvlat_t[:, kl, cs],
                                start=(kl == 0), stop=(kl == NLKV - 1),
                            )
                        nc.scalar.copy(kTn_t[:, h, cs], ps[:])
                    for stl in range(4):
                        st = 4 * sb + stl
                        ps = ps_p1.tile([128, 512], F32, tag="ch")
                        for kl in range(NLKV):
                            nc.tensor.matmul(
                                ps[:],
                                kvlat_t[:, kl,
                                        512 * sb + 128 * stl:
                                        512 * sb + 128 * (stl + 1)],
                                wuv_t[:, kl, :],
                                start=(kl == 0), stop=(kl == NLKV - 1),
                            )
                        nc.scalar.copy(v_t[:, st, :], ps[:])
                    for pr in range(2):
                        ps = ps_p1.tile([128, 512], F32, tag="ch")
                        for kl in range(NLKV):
                            nc.tensor.matmul(
                                ps[:], wkr_t[:, kl, 128 * pr:128 * (pr + 1)],
                                kvlat_t[:, kl, cs],
                                start=(kl == 0), stop=(kl == NLKV - 1),
                            )
                        _rope_apply(nc, p1tmp, ps, c4_t[:, cs], s4_t[:, cs],
                                    kTr_t[:, pr, cs])

            # ---------------- P2: attention + W_o ----------------
            PHASE_MARKS["P2"] = nc.next_id()
            with (
                tc.tile_pool(name="p2q", bufs=4) as p2q,
                tc.tile_pool(name="p2qr", bufs=2) as p2qr,
                tc.tile_pool(name="p2exp", bufs=2) as p2exp,
                tc.tile_pool(name="p2acc", bufs=2) as p2acc,
                tc.tile_pool(name="p2red", bufs=2) as p2red,
                tc.tile_pool(name="p2out", bufs=9) as p2out,
                tc.tile_pool(name="p2tmp", bufs=2) as p2tmp,
                tc.tile_pool(name="p2y", bufs=2) as p2y,
                tc.tile_pool(name="ps_sc", bufs=2, space="PSUM") as ps_sc,
                tc.tile_pool(name="ps_pv", bufs=2, space="PSUM") as ps_pv,
                tc.tile_pool(name="ps_pj", bufs=2, space="PSUM") as ps_pj,
            ):
                def emit_wo(out_tiles, cs):
                    for g in range(NDT // 4):
                        ystage = p2y.tile([128, 4, 512], BF16, tag="y")
                        for dtl in range(4):
                            dt = 4 * g + dtl
                            ps_y = ps_pj.tile([128, 512], F32, tag="pj")
                            for h in range(4):
                                nc.tensor.matmul(
                                    ps_y[:],
                                    wo_t[:, h, 128 * dt:128 * (dt + 1)],
                                    out_tiles[h][:],
                                    start=(h == 0), stop=(h == 3),
                                )
                            nc.vector.tensor_copy(ystage[:, dtl, :], ps_y[:])
                        nc.sync.dma_start(
                            col3(yT_d[512 * g:512 * (g + 1), cs]), ystage[:]
                        )

                prev_out = None
                prev_cs = None
                for qb in range(n_sb):
                    cs = slice(512 * qb, 512 * (qb + 1))
                    qr_tiles = []
                    for pr in range(2):
                        ps = ps_pj.tile([128, 512], F32, tag="pj")
                        for ql in range(NLQ):
                            nc.tensor.matmul(
                                ps[:], wqr_t[:, ql, 128 * pr:128 * (pr + 1)],
                                qlat_t[:, ql, cs],
                                start=(ql == 0), stop=(ql == NLQ - 1),
                            )
                        qr = p2qr.tile([128, 512], BF16, tag="qr")
                        _rope_apply(nc, p2tmp, ps, c4_t[:, cs], s4_t[:, cs],
                                    qr[:])
                        qr_tiles.append(qr)
                    qn_tiles = []
                    for h in range(4):
                        ps = ps_pj.tile([128, 512], F32, tag="pj")
                        for ql in range(NLQ):
                            nc.tensor.matmul(
                                ps[:], wuq_t[:, ql, 128 * h:128 * (h + 1)],
                                qlat_t[:, ql, cs],
                                start=(ql == 0), stop=(ql == NLQ - 1),
                            )
                        qn = p2q.tile([128, 512], BF16, tag="qn", name=f"qn{h}")
                        nc.scalar.copy(qn[:], ps[:])
                        qn_tiles.append(qn)
                    if prev_out is not None:
                        emit_wo(prev_out, prev_cs)
                    out_tiles = []
                    for h in range(4):
                        qn = qn_tiles[h]
                        qr = qr_tiles[h // 2]
                        pb = 64 * (h % 2)
                        nkt = 4 * (qb + 1)

                        def vstart(kt):
                            delta = 128 * kt - 512 * qb
                            return 0 if delta < 0 else min(delta, 256)

                        ps_o = ps_pv.tile([128, 512], F32, tag="pv")
                        dacc = p2acc.tile([128, 512], F32, tag="dacc")

                        def emit_pv(exp_pair, pk, ps_o=ps_o, h=h, nkt=nkt,
                                    vstart=vstart):
                            for j in (0, 1):
                                kt = 2 * pk + j
                                vs = vstart(kt)
                                nc.tensor.matmul(
                                    ps_o[:, vs:512],
                                    v_t[:, kt, 128 * h:128 * (h + 1)],
                                    exp_pair[:, 512 * j + vs:512 * (j + 1)],
                                    start=(kt == 0), stop=(kt == nkt - 1),
                                    skip_group_check=True,
                                )

                        npair = nkt // 2
                        pend = []   # (exp pair tile, pk) one pair behind
                        for pk in range(npair):
                            ps_s = ps_sc.tile([128, 1024], F32, tag="sc")
                            for j in (0, 1):
                                kt = 2 * pk + j
                                ks = slice(128 * kt, 128 * (kt + 1))
                                delta = 128 * kt - 512 * qb
                                diag = delta >= 0
                                vs = vstart(kt)
                                half = ps_s[:, 512 * j + vs:512 * (j + 1)]
                                nc.tensor.matmul(
                                    half, kTn_t[:, h, ks], qn[:, vs:512],
                                    start=True, stop=False,
                                    skip_group_check=True,
                                )
                                nc.tensor.matmul(
                                    half,
                                    kTr_t[pb:pb + 64, h // 2, ks],
                                    qr[pb:pb + 64, vs:512],
                                    start=False, stop=not diag,
                                    skip_group_check=True,
                                )
                                if diag:
                                    mcol = 384 + vs - delta
                                    nc.tensor.matmul(
                                        ps_s[:, 512 * j + vs:512 * j + vs + 256],
                                        id_t[:],
                                        mask_t[:, mcol:mcol + 256],
                                        start=False, stop=True,
                                        skip_group_check=True,
                                    )
                            vs0 = vstart(2 * pk)
                            vs1 = vstart(2 * pk + 1)
                            exp_t = p2exp.tile([128, 1024], BF16, tag="exp")
                            if vs0 == 0 and vs1 == 0:
                                nc.scalar.activation(
                                    exp_t[:], ps_s[:], AF.Exp, scale=SCALE)
                            else:
                                nc.scalar.activation(
                                    exp_t[:, vs0:512], ps_s[:, vs0:512],
                                    AF.Exp, scale=SCALE)
                                nc.scalar.activation(
                                    exp_t[:, 512 + vs1:1024],
                                    ps_s[:, 512 + vs1:1024],
                                    AF.Exp, scale=SCALE)
                            if pk == 0:
                                nc.vector.tensor_copy(
                                    dacc[:], exp_t[:, 0:512])
                            else:
                                nc.vector.tensor_add(
                                    dacc[:, vs0:512], dacc[:, vs0:512],
                                    exp_t[:, vs0:512])
                            nc.vector.tensor_add(
                                dacc[:, vs1:512], dacc[:, vs1:512],
                                exp_t[:, 512 + vs1:1024])
                            pend.append((exp_t, pk))
                            if len(pend) > 1:
                                emit_pv(*pend.pop(0))
                        for e in pend:
                            emit_pv(*e)
                        red = p2red.tile([128, 512], F32, tag="dred")
                        nc.gpsimd.partition_all_reduce(
                            red[:], dacc[:], 128, bass_isa.ReduceOp.add
                        )
                        nc.vector.reciprocal(red[:], red[:])
                        out_t = p2out.tile([128, 512], BF16, tag="outT")
                        nc.vector.tensor_mul(out_t[:], ps_o[:], red[:])
                        out_tiles.append(out_t)
                    prev_out, prev_cs = out_tiles, cs
                emit_wo(prev_out, prev_cs)
            persist_stack.close()

    nc.compile()
    return nc


def host_prep(inputs, S=S_FULL):
    """Build the 8 per-core input maps from the full problem inputs."""
    x = np.asarray(inputs["x"], np.float32)
    cosT = np.ascontiguousarray(np.asarray(inputs["rope_cos"], np.float32).T)
    sinT = np.ascontiguousarray(np.asarray(inputs["rope_sin"], np.float32).T)
    c4 = np.ascontiguousarray(np.concatenate([cosT, cosT, cosT, cosT], 0))
    s4 = np.ascontiguousarray(np.concatenate([-sinT, sinT, -sinT, sinT], 0))
    qw = np.asarray(inputs["q_norm_w"], np.float32)
    kvw = np.asarray(inputs["kv_norm_w"], np.float32)
    W_uq = np.asarray(inputs["W_uq"], np.float32) * qw[:, None]
    W_qr = np.asarray(inputs["W_qr"], np.float32) * qw[:, None]
    W_uk = np.asarray(inputs["W_uk"], np.float32) * kvw[:, None]
    W_kr = np.asarray(inputs["W_kr"], np.float32) * kvw[:, None]
    W_uv = np.asarray(inputs["W_uv"], np.float32) * kvw[:, None]
    W_o = np.asarray(inputs["W_o"], np.float32)
    W_dq = np.ascontiguousarray(np.asarray(inputs["W_dq"], np.float32)).astype(NPBF)
    W_dkv = np.ascontiguousarray(np.asarray(inputs["W_dkv"], np.float32)).astype(NPBF)

    cgrid = np.arange(896)[None, :] - 384
    igrid = np.arange(128)[:, None]
    mask_big = np.where(cgrid >= igrid, 0.0, MASK_NEG).astype(NPBF)
    ident = np.eye(128, dtype=np.float32).astype(NPBF)

    in_maps = []
    for c in range(NCORES):
        b, g = c // 4, c % 4
        hs = slice(4 * g * DN, 4 * (g + 1) * DN)
        hr = slice(4 * g * DR, 4 * (g + 1) * DR)
        in_maps.append(dict(
            xT=np.ascontiguousarray(x[b].T).astype(NPBF),
            W_dq=W_dq, W_dkv=W_dkv,
            Wuq=np.ascontiguousarray(W_uq[:, hs]).astype(NPBF),
            Wqr=np.ascontiguousarray(W_qr[:, hr]).astype(NPBF),
            Wuk=np.ascontiguousarray(W_uk[:, hs]).astype(NPBF),
            Wkr=np.ascontiguousarray(W_kr[:, hr]).astype(NPBF),
            Wuv=np.ascontiguousarray(W_uv[:, hs]).astype(NPBF),
            Wo=np.ascontiguousarray(W_o[512 * g:512 * (g + 1), :]).astype(NPBF),
            c4=c4, s4=s4, mask_big=mask_big, ident=ident,
            ones_col=np.ones((128, 1), NPBF),
        ))
    return in_maps


_NC_CACHE = {}


def kernel(**inputs) -> np.ndarray:
    S = np.asarray(inputs["x"]).shape[1]
    if S not in _NC_CACHE:
        _NC_CACHE[S] = build_nc(S)
    nc = _NC_CACHE[S]
    in_maps = host_prep(inputs, S)
    trace = bool(os.environ.get("MLA_TRACE"))
    res = run_bass_kernel_spmd(
        nc, in_maps, core_ids=list(range(NCORES)), trace=trace
    )
    if trace:
        print(f"HW exec time: {res.exec_time_ns} ns")
        print(f"trace: {res.instructions_and_trace[1] if res.instructions_and_trace else None}")
    y = np.empty((B, S, D), np.float32)
    for b in range(B):
        acc = res.results[4 * b]["yT"].astype(np.float32).copy()
        for g in range(1, 4):
            acc += res.results[4 * b + g]["yT"].astype(np.float32)
        y[b] = acc.T
    return y


# revision 27
# speedup vs baseline: 1.4657x; 1.1082x over previous
"""MultiHeadLatentAttention (MLA) Trainium2 Bass kernel, v2.

Problem: B=2, S=2048, D=2048, H=16 heads, d_nope=128, d_rope=64, d_head=128,
q_latent=768, kv_latent=512. Causal attention, rmsnorm'd latents, half-dim RoPE.

Sharding (8 cores): core c handles batch b=c//4 and head group g=c%4 (4 heads).
The small latent down-projections are replicated within each batch group;
W_uq/W_qr/W_uk/W_kr/W_uv are column-sharded by head; W_o row-sharded; the
4 partial outputs per batch are summed on the host.

v2 changes vs v1 (559us):
  * all matmul operands in bf16 (same PE rate as f32r in the cost model,
    half the DMA bytes and SBUF footprint; measured end-to-end rel err ~5e-3
    vs the 2e-2 gate).
  * latents stay resident in SBUF (no DRAM round trip).
  * ~40 large DMAs instead of 245 small ones (SP sequencer was 97% busy
    issuing DMAs in v1); W_o loaded once; x loaded in [128,8,512] halves.
  * rmsnorm r = sqrt(1/(mean+eps)) via ACT Copy -> DVE reciprocal -> ACT
    Sqrt, and softmax 1/den via DVE reciprocal + mul. v1 used Ln+Exp
    pairs, which thrashed the activation-function table (49
    LoadActFuncSet x 1.3us, most on the softmax critical path).
  * causal clipping: diagonal score/PV/mask matmuls only cover the
    unmasked column suffix (>=256 wide to stay at 1 cycle/row).
  * rmsnorm sumsq accumulated on the idle Pool engine (frees PE + ACT).
  * P2 runs a flattened (head, pair) software pipeline: PV matmuls trail
    scores by 2 pairs across head boundaries; W_o of the previous q-block
    is interleaved after the projections of the next.
"""
import math
import os
from contextlib import ExitStack

import numpy as np
import ml_dtypes

import concourse.bass as bass
import concourse.bass_isa as bass_isa
import concourse.bacc as bacc
import concourse.mybir as mybir
import concourse.tile as tile
from concourse.bass_utils import run_bass_kernel_spmd

F32 = mybir.dt.float32
F32R = mybir.dt.float32r
BF16 = mybir.dt.bfloat16
AF = mybir.ActivationFunctionType
NPBF = ml_dtypes.bfloat16

B, S_FULL, D = 2, 2048, 2048
H, DN, DR, DH = 16, 128, 64, 128
QL, KVL = 768, 512
EPS = 1e-6
SCALE = 1.0 / math.sqrt(DH)
MASK_NEG = -1e6
NCORES = 8
NKT = D // 128          # 16 contraction tiles over D
NLQ = QL // 128         # 6
NLKV = KVL // 128       # 4
NDT = D // 128          # 16 output D tiles


def _rope_apply(nc, pool, ps, c4s, s4s, out_ap):
    """Half-dim rope on a pair tile [128, 512] (h_even x1|x2 | h_odd x1|x2).

    out = rs * c4 + shuf(rs) * s4,  shuf swaps the 32-blocks within each 64.
    ps (PSUM f32) is staged to SBUF bf16 with one ACT copy first: that frees
    the PSUM bank in ~0.6us, and the remaining DVE ops run all-bf16-SBUF,
    hitting the 4x (copy) / 2x (mul,add) DVE fast paths.
    """
    rs = pool.tile([128, 512], BF16, tag="rope_rs")
    nc.scalar.copy(rs[:], ps[:])
    shuf = pool.tile([128, 512], BF16, tag="rope_shuf")
    nc.vector.tensor_copy(shuf[0:32, :], rs[32:64, :])
    nc.vector.tensor_copy(shuf[32:64, :], rs[0:32, :])
    nc.vector.tensor_copy(shuf[64:96, :], rs[96:128, :])
    nc.vector.tensor_copy(shuf[96:128, :], rs[64:96, :])
    t1 = pool.tile([128, 512], BF16, tag="rope_t1")
    nc.vector.tensor_mul(t1[:], rs[:], c4s)
    nc.vector.tensor_mul(shuf[:], shuf[:], s4s)
    nc.vector.tensor_add(out_ap, shuf[:], t1[:])


PHASE_MARKS = {}


def build_nc(S=S_FULL):
    assert S % 512 == 0
    n_sb = S // 512
    n_st = S // 128
    PHASE_MARKS.clear()

    nc = bacc.Bacc("TRN2", target_bir_lowering=False, debug=False,
                   num_devices=NCORES)

    xT_d = nc.dram_tensor("xT", [D, S], BF16, kind="ExternalInput")
    wdq_d = nc.dram_tensor("W_dq", [D, QL], BF16, kind="ExternalInput")
    wdkv_d = nc.dram_tensor("W_dkv", [D, KVL], BF16, kind="ExternalInput")
    wuq_d = nc.dram_tensor("Wuq", [QL, 512], BF16, kind="ExternalInput")
    wqr_d = nc.dram_tensor("Wqr", [QL, 256], BF16, kind="ExternalInput")
    wuk_d = nc.dram_tensor("Wuk", [KVL, 512], BF16, kind="ExternalInput")
    wkr_d = nc.dram_tensor("Wkr", [KVL, 256], BF16, kind="ExternalInput")
    wuv_d = nc.dram_tensor("Wuv", [KVL, 512], BF16, kind="ExternalInput")
    wo_d = nc.dram_tensor("Wo", [512, D], BF16, kind="ExternalInput")
    c4_d = nc.dram_tensor("c4", [128, S], BF16, kind="ExternalInput")
    s4_d = nc.dram_tensor("s4", [128, S], BF16, kind="ExternalInput")
    mask_d = nc.dram_tensor("mask_big", [128, 896], BF16, kind="ExternalInput")
    id_d = nc.dram_tensor("ident", [128, 128], BF16, kind="ExternalInput")
    yT_d = nc.dram_tensor("yT", [D, S], BF16, kind="ExternalOutput")

    def col3(dram_ap, p=128):
        # [R, C] dram slice -> [128, R//128, C] tiled AP
        return dram_ap.rearrange("(t p) c -> p t c", p=p)

    with tile.TileContext(nc) as tc:
        with (
            tc.tile_pool(name="const", bufs=1) as constp,
            tc.tile_pool(name="lat", bufs=1) as latp,
            tc.tile_pool(name="upw", bufs=1) as upw,
        ):
            mask_t = constp.tile([128, 896], BF16)
            id_t = constp.tile([128, 128], BF16)
            c4_t = constp.tile([128, S], BF16)
            s4_t = constp.tile([128, S], BF16)
            qlat_t = latp.tile([128, NLQ, S], BF16)
            kvlat_t = latp.tile([128, NLKV, S], BF16)
            wuk_t = upw.tile([128, NLKV, 512], BF16)
            wkr_t = upw.tile([128, NLKV, 256], BF16)
            wuv_t = upw.tile([128, NLKV, 512], BF16)
            wuq_t = upw.tile([128, NLQ, 512], BF16)
            wqr_t = upw.tile([128, NLQ, 256], BF16)
            wo_t = upw.tile([128, 4, D], BF16)

            # ---------------- P0: down-projections + rmsnorm ----------------
            PHASE_MARKS["P0"] = nc.next_id()
            with (
                tc.tile_pool(name="p0w", bufs=1) as p0w,
                tc.tile_pool(name="p0x", bufs=4) as p0x,
                tc.tile_pool(name="p0raw", bufs=2) as p0raw,
                tc.tile_pool(name="p0sq", bufs=1) as p0sq,
                tc.tile_pool(name="p0r", bufs=2) as p0r,
                tc.tile_pool(name="ps_ch", bufs=3, space="PSUM") as ps_ch,
            ):
                wdq_t = p0w.tile([128, NKT, QL], BF16)
                wdkv_t = p0w.tile([128, NKT, KVL], BF16)

                def xh_tiles(sb):
                    return [
                        p0x.tile([128, 8, 512], BF16, tag="xsb",
                                 name=f"xsb{sb}_{i}")
                        for i in range(2)
                    ]

                def load_x(sb, xh):
                    cs = slice(512 * sb, 512 * (sb + 1))
                    for i in range(2):
                        nc.sync.dma_start(
                            xh[i][:],
                            col3(xT_d[:, cs])[:, 8 * i:8 * (i + 1), :],
                        )

                # startup loads: feed the first down-projection chain in
                # small pieces so the PE starts ~3us in instead of ~10us.
                xh_cur = xh_tiles(0)
                x0 = col3(xT_d[:, 0:512])
                nc.sync.dma_start(wdq_t[:, 0:8, 0:128], col3(wdq_d[:, 0:128])[:, 0:8, :])
                nc.sync.dma_start(xh_cur[0][:, 0:4, :], x0[:, 0:4, :])
                nc.sync.dma_start(xh_cur[0][:, 4:8, :], x0[:, 4:8, :])
                nc.sync.dma_start(wdq_t[:, 8:16, 0:128], col3(wdq_d[:, 0:128])[:, 8:16, :])
                nc.sync.dma_start(xh_cur[1][:, 0:4, :], x0[:, 8:12, :])
                nc.sync.dma_start(xh_cur[1][:, 4:8, :], x0[:, 12:16, :])
                nc.sync.dma_start(wdq_t[:, :, 128:384], col3(wdq_d[:, 128:384]))
                nc.sync.dma_start(wdq_t[:, :, 384:768], col3(wdq_d[:, 384:768]))
                nc.sync.dma_start(wdkv_t[:], col3(wdkv_d[:]))
                nc.sync.dma_start(c4_t[:], c4_d[:])
                nc.sync.dma_start(s4_t[:], s4_d[:])
                nc.sync.dma_start(mask_t[:], mask_d[:])
                nc.sync.dma_start(id_t[:], id_d[:])

                for sb in range(n_sb):
                    cs = slice(512 * sb, 512 * (sb + 1))
                    xh = xh_cur
                    if sb + 1 < n_sb:
                        xh_cur = xh_tiles(sb + 1)
                        load_x(sb + 1, xh_cur)
                    for latname, w_t, nl, lat_t in (
                        ("q", wdq_t, NLQ, qlat_t),
                        ("kv", wdkv_t, NLKV, kvlat_t),
                    ):
                        raw = p0raw.tile([128, nl, 512], BF16,
                                         tag=f"raw{latname}",
                                         name=f"raw{latname}")
                        # sumsq accumulated on the (otherwise idle) Pool
                        # engine so neither PE nor ACT carries it.
                        sqa = p0sq.tile([128, 512], F32, tag=f"sqa{latname}",
                                        name=f"sqa{latname}")
                        sqt = p0sq.tile([128, 512], F32, tag=f"sqt{latname}",
                                        name=f"sqt{latname}")
                        for lt in range(nl):
                            ps = ps_ch.tile([128, 512], F32, tag="ch")
                            for kt in range(NKT):
                                nc.tensor.matmul(
                                    ps[:],
                                    w_t[:, kt, 128 * lt:128 * (lt + 1)],
                                    xh[kt // 8][:, kt % 8, :],
                                    start=(kt == 0), stop=(kt == NKT - 1),
                                )
                            nc.scalar.copy(raw[:, lt, :], ps[:])
                            if lt == 0:
                                nc.gpsimd.tensor_mul(
                                    sqa[:], raw[:, 0, :], raw[:, 0, :])
                            else:
                                nc.gpsimd.tensor_mul(
                                    sqt[:], raw[:, lt, :], raw[:, lt, :])
                                nc.gpsimd.tensor_add(sqa[:], sqa[:], sqt[:])
                        sumb = p0r.tile([128, 512], F32, tag="sumb")
                        nc.gpsimd.partition_all_reduce(
                            sumb[:], sqa[:], 128, bass_isa.ReduceOp.add)
                        # r = 1/sqrt(mean+eps) without touching the Exp/Ln
                        # activation tables: Copy(scale,bias) -> DVE
                        # reciprocal -> Sqrt (Sqrt shares a table set with
                        # Copy, so P0 needs no table reloads).
                        ms_t = p0r.tile([128, 512], F32, tag="msrow")
                        nc.scalar.activation(
                            ms_t[:], sumb[:], AF.Copy,
                            scale=1.0 / (128 * nl), bias=float(EPS),
                        )
                        inv_t = p0r.tile([128, 512], F32, tag="invrow")
                        nc.vector.reciprocal(inv_t[:], ms_t[:])
                        rsb = p0r.tile([128, 512], BF16, tag="rsb")
                        nc.scalar.activation(rsb[:], inv_t[:], AF.Sqrt)
                        for lt in range(nl):
                            nc.vector.tensor_mul(
                                lat_t[:, lt, cs], raw[:, lt, :], rsb[:])
                    if sb == 0:
                        # up-projection weights: needed from P1 on; queue the
                        # transfers behind the first x prefetch.
                        nc.sync.dma_start(wuk_t[:], col3(wuk_d[:]))
                        nc.sync.dma_start(wkr_t[:], col3(wkr_d[:]))
                        nc.sync.dma_start(wuv_t[:], col3(wuv_d[:]))
                        nc.sync.dma_start(wuq_t[:], col3(wuq_d[:]))
                        nc.sync.dma_start(wqr_t[:], col3(wqr_d[:]))
                        nc.sync.dma_start(wo_t[:], col3(wo_d[:]))

            # ---------------- P1: k/v up-projections ----------------
            PHASE_MARKS["P1"] = nc.next_id()
            persist_stack = ExitStack()
            kvp = persist_stack.enter_context(tc.tile_pool(name="kvp", bufs=1))
            kTn_t = kvp.tile([128, 4, S], BF16)    # nope k^T per head
            kTr_t = kvp.tile([128, 2, S], BF16)    # rope k^T per pair
            v_t = kvp.tile([128, n_st, 512], BF16)  # v natural
            with (
                tc.tile_pool(name="ps_p1", bufs=6, space="PSUM") as ps_p1,
                tc.tile_pool(name="p1tmp", bufs=2) as p1tmp,
            ):
                def kTn_chain(sb, h, cs):
                    ps = ps_p1.tile([128, 512], F32, tag="ch")
                    for kl in range(NLKV):
                        nc.tensor.matmul(
                            ps[:], wuk_t[:, kl, 128 * h:128 * (h + 1)],
                            kvlat_t[:, kl, cs],
                            start=(kl == 0), stop=(kl == NLKV - 1),
                        )
                    nc.scalar.copy(kTn_t[:, h, cs], ps[:])

                def kTr_chain(sb, pr, cs):
                    ps = ps_p1.tile([128, 512], F32, tag="ch")
                    for kl in range(NLKV):
                        nc.tensor.matmul(
                            ps[:], wkr_t[:, kl, 128 * pr:128 * (pr + 1)],
                            kvlat_t[:, kl, cs],
                            start=(kl == 0), stop=(kl == NLKV - 1),
                        )
                    _rope_apply(nc, p1tmp, ps, c4_t[:, cs], s4_t[:, cs],
                                kTr_t[:, pr, cs])

                def v_chain(sb, stl):
                    st = 4 * sb + stl
                    ps = ps_p1.tile([128, 512], F32, tag="ch")
                    for kl in range(NLKV):
                        nc.tensor.matmul(
                            ps[:],
                            kvlat_t[:, kl,
                                    512 * sb + 128 * stl:
                                    512 * sb + 128 * (stl + 1)],
                            wuv_t[:, kl, :],
                            start=(kl == 0), stop=(kl == NLKV - 1),
                        )
                    nc.scalar.copy(v_t[:, st, :], ps[:])

                # rope chains sit mid-sequence so their long DVE tails
                # overlap the surrounding matmul chains instead of stalling
                # the 3-buffer PSUM rotation (or the P2 start) behind them.
                for sb in range(n_sb):
                    cs = slice(512 * sb, 512 * (sb + 1))
                    kTn_chain(sb, 0, cs)
                    kTn_chain(sb, 1, cs)
                    kTr_chain(sb, 0, cs)
                    kTn_chain(sb, 2, cs)
                    kTn_chain(sb, 3, cs)
                    kTr_chain(sb, 1, cs)
                    for stl in range(4):
                        v_chain(sb, stl)

            # ---------------- P2: attention + W_o ----------------
            PHASE_MARKS["P2"] = nc.next_id()
            with (
                tc.tile_pool(name="p2q", bufs=4) as p2q,
                tc.tile_pool(name="p2qr", bufs=2) as p2qr,
                tc.tile_pool(name="p2exp", bufs=4) as p2exp,
                tc.tile_pool(name="p2acc", bufs=3) as p2acc,
                tc.tile_pool(name="p2red", bufs=2) as p2red,
                tc.tile_pool(name="p2out", bufs=9) as p2out,
                tc.tile_pool(name="p2tmp", bufs=2) as p2tmp,
                tc.tile_pool(name="p2y", bufs=2) as p2y,
                tc.tile_pool(name="ps_sc", bufs=2, space="PSUM") as ps_sc,
                tc.tile_pool(name="ps_pv", bufs=2, space="PSUM") as ps_pv,
                tc.tile_pool(name="ps_pj", bufs=2, space="PSUM") as ps_pj,
            ):
                def emit_wo(out_tiles, cs, alt_pool=False):
                    for g2 in range(NDT // 2):
                        ystage = p2y.tile([128, 2, 512], BF16, tag="y")
                        for dtl in range(2):
                            dt = 2 * g2 + dtl
                            if alt_pool and dt % 2 == 1:
                                ps_y = ps_pv.tile([128, 512], F32, tag="pv",
                                                  name="pswo")
                            else:
                                ps_y = ps_pj.tile([128, 512], F32, tag="pj")
                            for h in range(4):
                                nc.tensor.matmul(
                                    ps_y[:],
                                    wo_t[:, h, 128 * dt:128 * (dt + 1)],
                                    out_tiles[h][:],
                                    start=(h == 0), stop=(h == 3),
                                )
                            nc.scalar.copy(ystage[:, dtl, :], ps_y[:])
                        nc.sync.dma_start(
                            col3(yT_d[256 * g2:256 * (g2 + 1), cs]), ystage[:]
                        )

                prev_out = None
                prev_cs = None
                pend = []   # (emit_pv_fn, h, exp tile, pk) across q-blocks
                for qb in range(n_sb):
                    cs = slice(512 * qb, 512 * (qb + 1))
                    qr_tiles = []
                    for pr in range(2):
                        ps = ps_pj.tile([128, 512], F32, tag="pj")
                        for ql in range(NLQ):
                            nc.tensor.matmul(
                                ps[:], wqr_t[:, ql, 128 * pr:128 * (pr + 1)],
                                qlat_t[:, ql, cs],
                                start=(ql == 0), stop=(ql == NLQ - 1),
                            )
                        qr = p2qr.tile([128, 512], BF16, tag="qr")
                        _rope_apply(nc, p2tmp, ps, c4_t[:, cs], s4_t[:, cs],
                                    qr[:])
                        qr_tiles.append(qr)
                    qn_tiles = []
                    for h in range(4):
                        ps = ps_pj.tile([128, 512], F32, tag="pj")
                        for ql in range(NLQ):
                            nc.tensor.matmul(
                                ps[:], wuq_t[:, ql, 128 * h:128 * (h + 1)],
                                qlat_t[:, ql, cs],
                                start=(ql == 0), stop=(ql == NLQ - 1),
                            )
                        qn = p2q.tile([128, 512], BF16, tag="qn", name=f"qn{h}")
                        nc.scalar.copy(qn[:], ps[:])
                        qn_tiles.append(qn)

                    # flush the previous block's trailing PV/softmax units
                    # behind the (independent) projection chains, then its
                    # W_o — so the PE never idles on the softmax tail. On
                    # the last block, W_o(prev) instead runs after the
                    # scores as runway for the final softmax tails.
                    for e in pend:
                        e[0](*e[1:])
                    pend = []
                    last_qb = qb == n_sb - 1
                    if prev_out is not None and not last_qb:
                        emit_wo(prev_out, prev_cs)

                    nkt = 4 * (qb + 1)
                    npair = nkt // 2

                    def vstart(kt, qb=qb):
                        delta = 128 * kt - 512 * qb
                        return 0 if delta < 0 else min(delta, 256)

                    out_tiles = []
                    hstate = {}

                    def emit_pv(h, exp_pair, pk):
                        st = hstate[h]
                        for j in (0, 1):
                            kt = 2 * pk + j
                            vs = vstart(kt)
                            nc.tensor.matmul(
                                st["ps_o"][:, vs:512],
                                v_t[:, kt, 128 * h:128 * (h + 1)],
                                exp_pair[:, 512 * j + vs:512 * (j + 1)],
                                start=(kt == 0), stop=(kt == nkt - 1),
                                skip_group_check=True,
                            )
                        if pk == npair - 1:
                            red = p2red.tile([128, 512], F32, tag="dred")
                            nc.gpsimd.partition_all_reduce(
                                red[:], st["dacc"][:], 128,
                                bass_isa.ReduceOp.add)
                            nc.vector.reciprocal(red[:], red[:])
                            out_t = p2out.tile([128, 512], BF16, tag="outT")
                            nc.vector.tensor_mul(out_t[:], st["ps_o"][:],
                                                 red[:])
                            out_tiles.append(out_t)

                    # flattened (head, pair) pipeline: PV matmuls trail the
                    # score matmuls by two pairs, crossing head and q-block
                    # boundaries, so the PE never waits on an exp at a tail.
                    for h in range(4):
                        qn = qn_tiles[h]
                        qr = qr_tiles[h // 2]
                        pb = 64 * (h % 2)
                        hstate[h] = {
                            "ps_o": ps_pv.tile([128, 512], F32, tag="pv",
                                               name=f"pv{h}"),
                            "dacc": p2acc.tile([128, 512], F32, tag="dacc",
                                               name=f"dacc{h}"),
                        }
                        dacc = hstate[h]["dacc"]
                        for pk in range(npair):
                            ps_s = ps_sc.tile([128, 1024], F32, tag="sc")
                            for j in (0, 1):
                                kt = 2 * pk + j
                                ks = slice(128 * kt, 128 * (kt + 1))
                                delta = 128 * kt - 512 * qb
                                diag = delta >= 0
                                vs = vstart(kt)
                                half = ps_s[:, 512 * j + vs:512 * (j + 1)]
                                nc.tensor.matmul(
                                    half, kTn_t[:, h, ks], qn[:, vs:512],
                                    start=True, stop=False,
                                    skip_group_check=True,
                                )
                                nc.tensor.matmul(
                                    half,
                                    kTr_t[pb:pb + 64, h // 2, ks],
                                    qr[pb:pb + 64, vs:512],
                                    start=False, stop=not diag,
                                    skip_group_check=True,
                                )
                                if diag:
                                    mcol = 384 + vs - delta
                                    nc.tensor.matmul(
                                        ps_s[:, 512 * j + vs:512 * j + vs + 256],
                                        id_t[:],
                                        mask_t[:, mcol:mcol + 256],
                                        start=False, stop=True,
                                        skip_group_check=True,
                                    )
                            vs0 = vstart(2 * pk)
                            vs1 = vstart(2 * pk + 1)
                            exp_t = p2exp.tile([128, 1024], BF16, tag="exp")
                            if vs0 == 0 and vs1 == 0:
                                nc.scalar.activation(
                                    exp_t[:], ps_s[:], AF.Exp, scale=SCALE)
                            else:
                                nc.scalar.activation(
                                    exp_t[:, vs0:512], ps_s[:, vs0:512],
                                    AF.Exp, scale=SCALE)
                                nc.scalar.activation(
                                    exp_t[:, 512 + vs1:1024],
                                    ps_s[:, 512 + vs1:1024],
                                    AF.Exp, scale=SCALE)
                            if pk == 0:
                                nc.vector.tensor_copy(
                                    dacc[:], exp_t[:, 0:512])
                            else:
                                nc.vector.tensor_add(
                                    dacc[:, vs0:512], dacc[:, vs0:512],
                                    exp_t[:, vs0:512])
                            nc.vector.tensor_add(
                                dacc[:, vs1:512], dacc[:, vs1:512],
                                exp_t[:, 512 + vs1:1024])
                            pend.append((emit_pv, h, exp_t, pk))
                            if len(pend) > 3:
                                e = pend.pop(0)
                                e[0](*e[1:])
                    if last_qb and prev_out is not None:
                        emit_wo(prev_out, prev_cs)
                    prev_out, prev_cs = out_tiles, cs
                for e in pend:
                    e[0](*e[1:])
                emit_wo(prev_out, prev_cs, alt_pool=True)
            persist_stack.close()

    nc.compile()
    return nc


def host_prep(inputs, S=S_FULL):
    """Build the 8 per-core input maps from the full problem inputs."""
    x = np.asarray(inputs["x"], np.float32)
    cosT = np.ascontiguousarray(np.asarray(inputs["rope_cos"], np.float32).T)
    sinT = np.ascontiguousarray(np.asarray(inputs["rope_sin"], np.float32).T)
    c4 = np.ascontiguousarray(np.concatenate([cosT, cosT, cosT, cosT], 0)).astype(NPBF)
    s4 = np.ascontiguousarray(np.concatenate([-sinT, sinT, -sinT, sinT], 0)).astype(NPBF)
    qw = np.asarray(inputs["q_norm_w"], np.float32)
    kvw = np.asarray(inputs["kv_norm_w"], np.float32)
    W_uq = np.asarray(inputs["W_uq"], np.float32) * qw[:, None]
    W_qr = np.asarray(inputs["W_qr"], np.float32) * qw[:, None]
    W_uk = np.asarray(inputs["W_uk"], np.float32) * kvw[:, None]
    W_kr = np.asarray(inputs["W_kr"], np.float32) * kvw[:, None]
    W_uv = np.asarray(inputs["W_uv"], np.float32) * kvw[:, None]
    W_o = np.asarray(inputs["W_o"], np.float32)
    W_dq = np.ascontiguousarray(np.asarray(inputs["W_dq"], np.float32)).astype(NPBF)
    W_dkv = np.ascontiguousarray(np.asarray(inputs["W_dkv"], np.float32)).astype(NPBF)

    cgrid = np.arange(896)[None, :] - 384
    igrid = np.arange(128)[:, None]
    mask_big = np.where(cgrid >= igrid, 0.0, MASK_NEG).astype(NPBF)
    ident = np.eye(128, dtype=np.float32).astype(NPBF)

    in_maps = []
    for c in range(NCORES):
        b, g = c // 4, c % 4
        hs = slice(4 * g * DN, 4 * (g + 1) * DN)
        hr = slice(4 * g * DR, 4 * (g + 1) * DR)
        in_maps.append(dict(
            xT=np.ascontiguousarray(x[b].T).astype(NPBF),
            W_dq=W_dq, W_dkv=W_dkv,
            Wuq=np.ascontiguousarray(W_uq[:, hs]).astype(NPBF),
            Wqr=np.ascontiguousarray(W_qr[:, hr]).astype(NPBF),
            Wuk=np.ascontiguousarray(W_uk[:, hs]).astype(NPBF),
            Wkr=np.ascontiguousarray(W_kr[:, hr]).astype(NPBF),
            Wuv=np.ascontiguousarray(W_uv[:, hs]).astype(NPBF),
            Wo=np.ascontiguousarray(W_o[512 * g:512 * (g + 1), :]).astype(NPBF),
            c4=c4, s4=s4, mask_big=mask_big, ident=ident,
        ))
    return in_maps


_NC_CACHE = {}


def kernel(**inputs) -> np.ndarray:
    S = np.asarray(inputs["x"]).shape[1]
    if S not in _NC_CACHE:
        _NC_CACHE[S] = build_nc(S)
    nc = _NC_CACHE[S]
    in_maps = host_prep(inputs, S)
    trace = bool(os.environ.get("MLA_TRACE"))
    res = run_bass_kernel_spmd(
        nc, in_maps, core_ids=list(range(NCORES)), trace=trace
    )
    if trace:
        print(f"HW exec time: {res.exec_time_ns} ns")
        print(f"trace: {res.instructions_and_trace[1] if res.instructions_and_trace else None}")
    y = np.empty((B, S, D), np.float32)
    for b in range(B):
        acc = res.results[4 * b]["yT"].astype(np.float32).copy()
        for g in range(1, 4):
            acc += res.results[4 * b + g]["yT"].astype(np.float32)
        y[b] = acc.T
    return y


# revision 35
# speedup vs baseline: 1.4847x; 1.0130x over previous
"""MultiHeadLatentAttention (MLA) Trainium2 Bass kernel, v2.

Problem: B=2, S=2048, D=2048, H=16 heads, d_nope=128, d_rope=64, d_head=128,
q_latent=768, kv_latent=512. Causal attention, rmsnorm'd latents, half-dim RoPE.

Sharding (8 cores): core c handles batch b=c//4 and head group g=c%4 (4 heads).
The small latent down-projections are replicated within each batch group;
W_uq/W_qr/W_uk/W_kr/W_uv are column-sharded by head; W_o row-sharded; the
4 partial outputs per batch are summed on the host.

v2 changes vs v1 (559us -> 377us under the InstructionCostModel timeline):
  * all matmul operands in bf16 (same PE rate as f32r in the cost model,
    half the DMA bytes and SBUF footprint; measured end-to-end rel err
    ~6e-3 vs the 2e-2 gate).
  * latents stay resident in SBUF (no DRAM round trip).
  * ~40 large DMAs instead of 245 small ones (the SP sequencer was 97%
    busy issuing DMAs in v1, with head-of-line blocking on input waits);
    W_o loaded once; x loaded in quarter/half blocks sized so the first
    down-projection chain starts ~4us in.
  * rmsnorm r = sqrt(1/(mean+eps)) via ACT Copy -> DVE reciprocal -> ACT
    Sqrt, and softmax 1/den via DVE reciprocal + mul. v1 used Ln+Exp
    pairs, which thrashed the activation-function table (49
    LoadActFuncSet x 1.3us, most on the softmax critical path). The
    only table switch left is Sqrt-set -> Exp-set at the P0/P2 boundary.
  * causal clipping: diagonal score/PV/mask matmuls, their exps and the
    denominator adds only cover the unmasked column suffix (kept >=256
    wide so fp32r/bf16 stay at 1 cycle/row).
  * rmsnorm sumsq accumulated on the (otherwise idle) Pool engine.
  * rope-applies stage PSUM to bf16 SBUF with one ACT copy (frees the
    PSUM bank in ~0.6us) and run the shuffle/muls all-bf16-SBUF, hitting
    the DVE 4x/2x fast paths.
  * P2 runs one flattened (head, pair) software pipeline per q-block: PV
    matmuls trail the score matmuls by 4 pairs, crossing head AND
    q-block boundaries; each block's trailing PV/softmax units and the
    previous block's W_o are emitted behind the next block's projection
    chains, so the PE never idles on a softmax tail. The last block's
    W_o runs after the final scores as tail runway.

PE occupancy in the timeline sim is ~94% (355us busy / 377us total);
the residual is the DMA-bound startup (~8.5us), the output drain
(~5.5us), and ~8us of scattered sub-us stalls.
"""
import math
import os
from contextlib import ExitStack

import numpy as np
import ml_dtypes

import concourse.bass as bass
import concourse.bass_isa as bass_isa
import concourse.bacc as bacc
import concourse.mybir as mybir
import concourse.tile as tile
from concourse.bass_utils import run_bass_kernel_spmd

F32 = mybir.dt.float32
F32R = mybir.dt.float32r
BF16 = mybir.dt.bfloat16
AF = mybir.ActivationFunctionType
NPBF = ml_dtypes.bfloat16

B, S_FULL, D = 2, 2048, 2048
H, DN, DR, DH = 16, 128, 64, 128
QL, KVL = 768, 512
EPS = 1e-6
SCALE = 1.0 / math.sqrt(DH)
MASK_NEG = -1e6
NCORES = 8
NKT = D // 128          # 16 contraction tiles over D
NLQ = QL // 128         # 6
NLKV = KVL // 128       # 4
NDT = D // 128          # 16 output D tiles


def _rope_apply(nc, pool, ps, c4s, s4s, out_ap):
    """Half-dim rope on a pair tile [128, 512] (h_even x1|x2 | h_odd x1|x2).

    out = rs * c4 + shuf(rs) * s4,  shuf swaps the 32-blocks within each 64.
    ps (PSUM f32) is staged to SBUF bf16 with one ACT copy first: that frees
    the PSUM bank in ~0.6us, and the remaining DVE ops run all-bf16-SBUF,
    hitting the 4x (copy) / 2x (mul,add) DVE fast paths.
    """
    rs = pool.tile([128, 512], BF16, tag="rope_rs")
    nc.scalar.copy(rs[:], ps[:])
    shuf = pool.tile([128, 512], BF16, tag="rope_shuf")
    nc.vector.tensor_copy(shuf[0:32, :], rs[32:64, :])
    nc.vector.tensor_copy(shuf[32:64, :], rs[0:32, :])
    nc.vector.tensor_copy(shuf[64:96, :], rs[96:128, :])
    nc.vector.tensor_copy(shuf[96:128, :], rs[64:96, :])
    t1 = pool.tile([128, 512], BF16, tag="rope_t1")
    nc.vector.tensor_mul(t1[:], rs[:], c4s)
    nc.vector.tensor_mul(shuf[:], shuf[:], s4s)
    nc.vector.tensor_add(out_ap, shuf[:], t1[:])


PHASE_MARKS = {}


def build_nc(S=S_FULL):
    assert S % 512 == 0
    n_sb = S // 512
    n_st = S // 128
    PHASE_MARKS.clear()

    nc = bacc.Bacc("TRN2", target_bir_lowering=False, debug=False,
                   num_devices=NCORES)

    xT_d = nc.dram_tensor("xT", [D, S], BF16, kind="ExternalInput")
    wdq_d = nc.dram_tensor("W_dq", [D, QL], BF16, kind="ExternalInput")
    wdkv_d = nc.dram_tensor("W_dkv", [D, KVL], BF16, kind="ExternalInput")
    wuq_d = nc.dram_tensor("Wuq", [QL, 512], BF16, kind="ExternalInput")
    wqr_d = nc.dram_tensor("Wqr", [QL, 256], BF16, kind="ExternalInput")
    wuk_d = nc.dram_tensor("Wuk", [KVL, 512], BF16, kind="ExternalInput")
    wkr_d = nc.dram_tensor("Wkr", [KVL, 256], BF16, kind="ExternalInput")
    wuv_d = nc.dram_tensor("Wuv", [KVL, 512], BF16, kind="ExternalInput")
    wo_d = nc.dram_tensor("Wo", [512, D], BF16, kind="ExternalInput")
    c4_d = nc.dram_tensor("c4", [128, S], BF16, kind="ExternalInput")
    s4_d = nc.dram_tensor("s4", [128, S], BF16, kind="ExternalInput")
    mask_d = nc.dram_tensor("mask_big", [128, 896], BF16, kind="ExternalInput")
    id_d = nc.dram_tensor("ident", [128, 128], BF16, kind="ExternalInput")
    yT_d = nc.dram_tensor("yT", [D, S], BF16, kind="ExternalOutput")

    def col3(dram_ap, p=128):
        # [R, C] dram slice -> [128, R//128, C] tiled AP
        return dram_ap.rearrange("(t p) c -> p t c", p=p)

    with tile.TileContext(nc) as tc:
        with (
            tc.tile_pool(name="const", bufs=1) as constp,
            tc.tile_pool(name="lat", bufs=1) as latp,
            tc.tile_pool(name="upw", bufs=1) as upw,
        ):
            mask_t = constp.tile([128, 896], BF16)
            id_t = constp.tile([128, 128], BF16)
            c4_t = constp.tile([128, S], BF16)
            s4_t = constp.tile([128, S], BF16)
            qlat_t = latp.tile([128, NLQ, S], BF16)
            kvlat_t = latp.tile([128, NLKV, S], BF16)
            wuk_t = upw.tile([128, NLKV, 512], BF16)
            wkr_t = upw.tile([128, NLKV, 256], BF16)
            wuv_t = upw.tile([128, NLKV, 512], BF16)
            wuq_t = upw.tile([128, NLQ, 512], BF16)
            wqr_t = upw.tile([128, NLQ, 256], BF16)
            wo_t = upw.tile([128, 4, D], BF16)

            # ---------------- P0: down-projections + rmsnorm ----------------
            PHASE_MARKS["P0"] = nc.next_id()
            with (
                tc.tile_pool(name="p0w", bufs=1) as p0w,
                tc.tile_pool(name="p0x", bufs=4) as p0x,
                tc.tile_pool(name="p0raw", bufs=2) as p0raw,
                tc.tile_pool(name="p0sq", bufs=1) as p0sq,
                tc.tile_pool(name="p0r", bufs=2) as p0r,
                tc.tile_pool(name="ps_ch", bufs=3, space="PSUM") as ps_ch,
            ):
                wdq_t = p0w.tile([128, NKT, QL], BF16)
                wdkv_t = p0w.tile([128, NKT, KVL], BF16)

                def xh_tiles(sb):
                    return [
                        p0x.tile([128, 8, 512], BF16, tag="xsb",
                                 name=f"xsb{sb}_{i}")
                        for i in range(2)
                    ]

                def load_x(sb, xh):
                    cs = slice(512 * sb, 512 * (sb + 1))
                    for i in range(2):
                        nc.sync.dma_start(
                            xh[i][:],
                            col3(xT_d[:, cs])[:, 8 * i:8 * (i + 1), :],
                        )

                # startup loads: feed the first down-projection chain in
                # small pieces so the PE starts ~3us in instead of ~10us.
                xh_cur = xh_tiles(0)
                x0 = col3(xT_d[:, 0:512])
                nc.sync.dma_start(wdq_t[:, 0:8, 0:128], col3(wdq_d[:, 0:128])[:, 0:8, :])
                nc.sync.dma_start(xh_cur[0][:, 0:4, :], x0[:, 0:4, :])
                nc.sync.dma_start(xh_cur[0][:, 4:8, :], x0[:, 4:8, :])
                nc.sync.dma_start(wdq_t[:, 8:16, 0:128], col3(wdq_d[:, 0:128])[:, 8:16, :])
                nc.sync.dma_start(xh_cur[1][:, 0:4, :], x0[:, 8:12, :])
                nc.sync.dma_start(xh_cur[1][:, 4:8, :], x0[:, 12:16, :])
                nc.sync.dma_start(wdq_t[:, :, 128:384], col3(wdq_d[:, 128:384]))
                nc.sync.dma_start(wdq_t[:, :, 384:768], col3(wdq_d[:, 384:768]))
                nc.sync.dma_start(wdkv_t[:], col3(wdkv_d[:]))
                nc.sync.dma_start(c4_t[:], c4_d[:])
                nc.sync.dma_start(s4_t[:], s4_d[:])
                nc.sync.dma_start(mask_t[:], mask_d[:])
                nc.sync.dma_start(id_t[:], id_d[:])

                for sb in range(n_sb):
                    cs = slice(512 * sb, 512 * (sb + 1))
                    xh = xh_cur
                    if sb + 1 < n_sb:
                        xh_cur = xh_tiles(sb + 1)
                        load_x(sb + 1, xh_cur)
                    for latname, w_t, nl, lat_t in (
                        ("q", wdq_t, NLQ, qlat_t),
                        ("kv", wdkv_t, NLKV, kvlat_t),
                    ):
                        raw = p0raw.tile([128, nl, 512], BF16,
                                         tag=f"raw{latname}",
                                         name=f"raw{latname}")
                        # sumsq accumulated on the (otherwise idle) Pool
                        # engine so neither PE nor ACT carries it.
                        sqa = p0sq.tile([128, 512], F32, tag=f"sqa{latname}",
                                        name=f"sqa{latname}")
                        sqt = p0sq.tile([128, 512], F32, tag=f"sqt{latname}",
                                        name=f"sqt{latname}")
                        for lt in range(nl):
                            ps = ps_ch.tile([128, 512], F32, tag="ch")
                            for kt in range(NKT):
                                nc.tensor.matmul(
                                    ps[:],
                                    w_t[:, kt, 128 * lt:128 * (lt + 1)],
                                    xh[kt // 8][:, kt % 8, :],
                                    start=(kt == 0), stop=(kt == NKT - 1),
                                )
                            nc.scalar.copy(raw[:, lt, :], ps[:])
                            if lt == 0:
                                nc.gpsimd.tensor_mul(
                                    sqa[:], raw[:, 0, :], raw[:, 0, :])
                            else:
                                nc.gpsimd.tensor_mul(
                                    sqt[:], raw[:, lt, :], raw[:, lt, :])
                                nc.gpsimd.tensor_add(sqa[:], sqa[:], sqt[:])
                        sumb = p0r.tile([128, 512], F32, tag="sumb")
                        nc.gpsimd.partition_all_reduce(
                            sumb[:], sqa[:], 128, bass_isa.ReduceOp.add)
                        # r = 1/sqrt(mean+eps) without touching the Exp/Ln
                        # activation tables: Copy(scale,bias) -> DVE
                        # reciprocal -> Sqrt (Sqrt shares a table set with
                        # Copy, so P0 needs no table reloads).
                        ms_t = p0r.tile([128, 512], F32, tag="msrow")
                        nc.scalar.activation(
                            ms_t[:], sumb[:], AF.Copy,
                            scale=1.0 / (128 * nl), bias=float(EPS),
                        )
                        inv_t = p0r.tile([128, 512], F32, tag="invrow")
                        nc.vector.reciprocal(inv_t[:], ms_t[:])
                        rsb = p0r.tile([128, 512], BF16, tag="rsb")
                        nc.scalar.activation(rsb[:], inv_t[:], AF.Sqrt)
                        for lt in range(nl):
                            nc.vector.tensor_mul(
                                lat_t[:, lt, cs], raw[:, lt, :], rsb[:])
                    if sb == 0:
                        # up-projection weights: needed from P1 on; queue the
                        # transfers behind the first x prefetch.
                        nc.sync.dma_start(wuk_t[:], col3(wuk_d[:]))
                        nc.sync.dma_start(wkr_t[:], col3(wkr_d[:]))
                        nc.sync.dma_start(wuv_t[:], col3(wuv_d[:]))
                        nc.sync.dma_start(wuq_t[:], col3(wuq_d[:]))
                        nc.sync.dma_start(wqr_t[:], col3(wqr_d[:]))
                        nc.sync.dma_start(wo_t[:], col3(wo_d[:]))

            # ---------------- P1: k/v up-projections ----------------
            PHASE_MARKS["P1"] = nc.next_id()
            persist_stack = ExitStack()
            kvp = persist_stack.enter_context(tc.tile_pool(name="kvp", bufs=1))
            kTn_t = kvp.tile([128, 4, S], BF16)    # nope k^T per head
            kTr_t = kvp.tile([128, 2, S], BF16)    # rope k^T per pair
            v_t = kvp.tile([128, n_st, 512], BF16)  # v natural
            with (
                tc.tile_pool(name="ps_p1", bufs=6, space="PSUM") as ps_p1,
                tc.tile_pool(name="p1tmp", bufs=2) as p1tmp,
            ):
                def kTn_chain(sb, h, cs):
                    ps = ps_p1.tile([128, 512], F32, tag="ch")
                    for kl in range(NLKV):
                        nc.tensor.matmul(
                            ps[:], wuk_t[:, kl, 128 * h:128 * (h + 1)],
                            kvlat_t[:, kl, cs],
                            start=(kl == 0), stop=(kl == NLKV - 1),
                        )
                    nc.scalar.copy(kTn_t[:, h, cs], ps[:])

                def kTr_chain(sb, pr, cs):
                    ps = ps_p1.tile([128, 512], F32, tag="ch")
                    for kl in range(NLKV):
                        nc.tensor.matmul(
                            ps[:], wkr_t[:, kl, 128 * pr:128 * (pr + 1)],
                            kvlat_t[:, kl, cs],
                            start=(kl == 0), stop=(kl == NLKV - 1),
                        )
                    _rope_apply(nc, p1tmp, ps, c4_t[:, cs], s4_t[:, cs],
                                kTr_t[:, pr, cs])

                def v_chain(sb, stl):
                    st = 4 * sb + stl
                    ps = ps_p1.tile([128, 512], F32, tag="ch")
                    for kl in range(NLKV):
                        nc.tensor.matmul(
                            ps[:],
                            kvlat_t[:, kl,
                                    512 * sb + 128 * stl:
                                    512 * sb + 128 * (stl + 1)],
                            wuv_t[:, kl, :],
                            start=(kl == 0), stop=(kl == NLKV - 1),
                        )
                    nc.scalar.copy(v_t[:, st, :], ps[:])

                # rope chains sit mid-sequence so their long DVE tails
                # overlap the surrounding matmul chains instead of stalling
                # the 3-buffer PSUM rotation (or the P2 start) behind them.
                for sb in range(n_sb):
                    cs = slice(512 * sb, 512 * (sb + 1))
                    kTn_chain(sb, 0, cs)
                    kTn_chain(sb, 1, cs)
                    kTr_chain(sb, 0, cs)
                    kTn_chain(sb, 2, cs)
                    kTn_chain(sb, 3, cs)
                    kTr_chain(sb, 1, cs)
                    for stl in range(4):
                        v_chain(sb, stl)

            # ---------------- P2: attention + W_o ----------------
            PHASE_MARKS["P2"] = nc.next_id()
            with (
                tc.tile_pool(name="p2q", bufs=4) as p2q,
                tc.tile_pool(name="p2qr", bufs=2) as p2qr,
                tc.tile_pool(name="p2exp", bufs=5) as p2exp,
                tc.tile_pool(name="p2acc", bufs=3) as p2acc,
                tc.tile_pool(name="p2red", bufs=2) as p2red,
                tc.tile_pool(name="p2out", bufs=9) as p2out,
                tc.tile_pool(name="p2tmp", bufs=2) as p2tmp,
                tc.tile_pool(name="p2y", bufs=2) as p2y,
                tc.tile_pool(name="ps_sc", bufs=2, space="PSUM") as ps_sc,
                tc.tile_pool(name="ps_pv", bufs=2, space="PSUM") as ps_pv,
                tc.tile_pool(name="ps_pj", bufs=2, space="PSUM") as ps_pj,
            ):
                def emit_wo(out_tiles, cs, alt_pool=False):
                    for g2 in range(NDT // 2):
                        ystage = p2y.tile([128, 2, 512], BF16, tag="y")
                        for dtl in range(2):
                            dt = 2 * g2 + dtl
                            if alt_pool and dt % 2 == 1:
                                ps_y = ps_pv.tile([128, 512], F32, tag="pv",
                                                  name="pswo")
                            else:
                                ps_y = ps_pj.tile([128, 512], F32, tag="pj")
                            for h in range(4):
                                nc.tensor.matmul(
                                    ps_y[:],
                                    wo_t[:, h, 128 * dt:128 * (dt + 1)],
                                    out_tiles[h][:],
                                    start=(h == 0), stop=(h == 3),
                                )
                            if alt_pool and dtl == 1:
                                nc.vector.tensor_copy(ystage[:, dtl, :],
                                                      ps_y[:])
                            else:
                                nc.scalar.copy(ystage[:, dtl, :], ps_y[:])
                        nc.sync.dma_start(
                            col3(yT_d[256 * g2:256 * (g2 + 1), cs]), ystage[:]
                        )

                prev_out = None
                prev_cs = None
                pend = []   # (emit_pv_fn, h, exp tile, pk) across q-blocks
                for qb in range(n_sb):
                    cs = slice(512 * qb, 512 * (qb + 1))
                    qr_tiles = []
                    for pr in range(2):
                        ps = ps_pj.tile([128, 512], F32, tag="pj")
                        for ql in range(NLQ):
                            nc.tensor.matmul(
                                ps[:], wqr_t[:, ql, 128 * pr:128 * (pr + 1)],
                                qlat_t[:, ql, cs],
                                start=(ql == 0), stop=(ql == NLQ - 1),
                            )
                        qr = p2qr.tile([128, 512], BF16, tag="qr")
                        _rope_apply(nc, p2tmp, ps, c4_t[:, cs], s4_t[:, cs],
                                    qr[:])
                        qr_tiles.append(qr)
                    qn_tiles = []
                    for h in range(4):
                        ps = ps_pj.tile([128, 512], F32, tag="pj")
                        for ql in range(NLQ):
                            nc.tensor.matmul(
                                ps[:], wuq_t[:, ql, 128 * h:128 * (h + 1)],
                                qlat_t[:, ql, cs],
                                start=(ql == 0), stop=(ql == NLQ - 1),
                            )
                        qn = p2q.tile([128, 512], BF16, tag="qn", name=f"qn{h}")
                        nc.scalar.copy(qn[:], ps[:])
                        qn_tiles.append(qn)

                    # flush the previous block's trailing PV/softmax units
                    # behind the (independent) projection chains, then its
                    # W_o — so the PE never idles on the softmax tail. On
                    # the last block, W_o(prev) instead runs after the
                    # scores as runway for the final softmax tails.
                    for e in pend:
                        e[0](*e[1:])
                    pend = []
                    last_qb = qb == n_sb - 1
                    if prev_out is not None and not last_qb:
                        emit_wo(prev_out, prev_cs)

                    nkt = 4 * (qb + 1)
                    npair = nkt // 2

                    def vstart(kt, qb=qb):
                        delta = 128 * kt - 512 * qb
                        return 0 if delta < 0 else min(delta, 256)

                    out_tiles = []
                    hstate = {}

                    def emit_pv(h, exp_pair, pk):
                        st = hstate[h]
                        for j in (0, 1):
                            kt = 2 * pk + j
                            vs = vstart(kt)
                            nc.tensor.matmul(
                                st["ps_o"][:, vs:512],
                                v_t[:, kt, 128 * h:128 * (h + 1)],
                                exp_pair[:, 512 * j + vs:512 * (j + 1)],
                                start=(kt == 0), stop=(kt == nkt - 1),
                                skip_group_check=True,
                            )
                        if pk == npair - 1:
                            red = p2red.tile([128, 512], F32, tag="dred")
                            nc.gpsimd.partition_all_reduce(
                                red[:], st["dacc"][:], 128,
                                bass_isa.ReduceOp.add)
                            nc.vector.reciprocal(red[:], red[:])
                            out_t = p2out.tile([128, 512], BF16, tag="outT")
                            nc.vector.tensor_mul(out_t[:], st["ps_o"][:],
                                                 red[:])
                            out_tiles.append(out_t)

                    # flattened (head, pair) pipeline: PV matmuls trail the
                    # score matmuls by two pairs, crossing head and q-block
                    # boundaries, so the PE never waits on an exp at a tail.
                    for h in range(4):
                        qn = qn_tiles[h]
                        qr = qr_tiles[h // 2]
                        pb = 64 * (h % 2)
                        hstate[h] = {
                            "ps_o": ps_pv.tile([128, 512], F32, tag="pv",
                                               name=f"pv{h}"),
                            "dacc": p2acc.tile([128, 512], F32, tag="dacc",
                                               name=f"dacc{h}"),
                        }
                        dacc = hstate[h]["dacc"]
                        for pk in range(npair):
                            ps_s = ps_sc.tile([128, 1024], F32, tag="sc")
                            for j in (0, 1):
                                kt = 2 * pk + j
                                ks = slice(128 * kt, 128 * (kt + 1))
                                delta = 128 * kt - 512 * qb
                                diag = delta >= 0
                                vs = vstart(kt)
                                half = ps_s[:, 512 * j + vs:512 * (j + 1)]
                                nc.tensor.matmul(
                                    half, kTn_t[:, h, ks], qn[:, vs:512],
                                    start=True, stop=False,
                                    skip_group_check=True,
                                )
                                nc.tensor.matmul(
                                    half,
                                    kTr_t[pb:pb + 64, h // 2, ks],
                                    qr[pb:pb + 64, vs:512],
                                    start=False, stop=not diag,
                                    skip_group_check=True,
                                )
                                if diag:
                                    mcol = 384 + vs - delta
                                    nc.tensor.matmul(
                                        ps_s[:, 512 * j + vs:512 * j + vs + 256],
                                        id_t[:],
                                        mask_t[:, mcol:mcol + 256],
                                        start=False, stop=True,
                                        skip_group_check=True,
                                    )
                            vs0 = vstart(2 * pk)
                            vs1 = vstart(2 * pk + 1)
                            exp_t = p2exp.tile([128, 1024], BF16, tag="exp")
                            if vs0 == 0 and vs1 == 0:
                                nc.scalar.activation(
                                    exp_t[:], ps_s[:], AF.Exp, scale=SCALE)
                            elif vs0 == vs1:
                                ps3 = ps_s[:].rearrange(
                                    "p (a c) -> p a c", a=2)
                                ex3 = exp_t[:].rearrange(
                                    "p (a c) -> p a c", a=2)
                                nc.scalar.activation(
                                    ex3[:, :, vs0:512], ps3[:, :, vs0:512],
                                    AF.Exp, scale=SCALE)
                            else:
                                nc.scalar.activation(
                                    exp_t[:, vs0:512], ps_s[:, vs0:512],
                                    AF.Exp, scale=SCALE)
                                nc.scalar.activation(
                                    exp_t[:, 512 + vs1:1024],
                                    ps_s[:, 512 + vs1:1024],
                                    AF.Exp, scale=SCALE)
                            if pk == 0:
                                nc.vector.tensor_copy(
                                    dacc[:], exp_t[:, 0:512])
                            else:
                                nc.vector.tensor_add(
                                    dacc[:, vs0:512], dacc[:, vs0:512],
                                    exp_t[:, vs0:512])
                            nc.vector.tensor_add(
                                dacc[:, vs1:512], dacc[:, vs1:512],
                                exp_t[:, 512 + vs1:1024])
                            pend.append((emit_pv, h, exp_t, pk))
                            if len(pend) > 4:
                                e = pend.pop(0)
                                e[0](*e[1:])
                    if last_qb and prev_out is not None:
                        emit_wo(prev_out, prev_cs)
                    prev_out, prev_cs = out_tiles, cs
                for e in pend:
                    e[0](*e[1:])
                emit_wo(prev_out, prev_cs, alt_pool=True)
            persist_stack.close()

    nc.compile()
    return nc


def host_prep(inputs, S=S_FULL):
    """Build the 8 per-core input maps from the full problem inputs."""
    x = np.asarray(inputs["x"], np.float32)
    cosT = np.ascontiguousarray(np.asarray(inputs["rope_cos"], np.float32).T)
    sinT = np.ascontiguousarray(np.asarray(inputs["rope_sin"], np.float32).T)
    c4 = np.ascontiguousarray(np.concatenate([cosT, cosT, cosT, cosT], 0)).astype(NPBF)
    s4 = np.ascontiguousarray(np.concatenate([-sinT, sinT, -sinT, sinT], 0)).astype(NPBF)
    qw = np.asarray(inputs["q_norm_w"], np.float32)
    kvw = np.asarray(inputs["kv_norm_w"], np.float32)
    W_uq = np.asarray(inputs["W_uq"], np.float32) * qw[:, None]
    W_qr = np.asarray(inputs["W_qr"], np.float32) * qw[:, None]
    W_uk = np.asarray(inputs["W_uk"], np.float32) * kvw[:, None]
    W_kr = np.asarray(inputs["W_kr"], np.float32) * kvw[:, None]
    W_uv = np.asarray(inputs["W_uv"], np.float32) * kvw[:, None]
    W_o = np.asarray(inputs["W_o"], np.float32)
    W_dq = np.ascontiguousarray(np.asarray(inputs["W_dq"], np.float32)).astype(NPBF)
    W_dkv = np.ascontiguousarray(np.asarray(inputs["W_dkv"], np.float32)).astype(NPBF)

    cgrid = np.arange(896)[None, :] - 384
    igrid = np.arange(128)[:, None]
    mask_big = np.where(cgrid >= igrid, 0.0, MASK_NEG).astype(NPBF)
    ident = np.eye(128, dtype=np.float32).astype(NPBF)

    in_maps = []
    for c in range(NCORES):
        b, g = c // 4, c % 4
        hs = slice(4 * g * DN, 4 * (g + 1) * DN)
        hr = slice(4 * g * DR, 4 * (g + 1) * DR)
        in_maps.append(dict(
            xT=np.ascontiguousarray(x[b].T).astype(NPBF),
            W_dq=W_dq, W_dkv=W_dkv,
            Wuq=np.ascontiguousarray(W_uq[:, hs]).astype(NPBF),
            Wqr=np.ascontiguousarray(W_qr[:, hr]).astype(NPBF),
            Wuk=np.ascontiguousarray(W_uk[:, hs]).astype(NPBF),
            Wkr=np.ascontiguousarray(W_kr[:, hr]).astype(NPBF),
            Wuv=np.ascontiguousarray(W_uv[:, hs]).astype(NPBF),
            Wo=np.ascontiguousarray(W_o[512 * g:512 * (g + 1), :]).astype(NPBF),
            c4=c4, s4=s4, mask_big=mask_big, ident=ident,
        ))
    return in_maps


_NC_CACHE = {}


def kernel(**inputs) -> np.ndarray:
    S = np.asarray(inputs["x"]).shape[1]
    if S not in _NC_CACHE:
        _NC_CACHE[S] = build_nc(S)
    nc = _NC_CACHE[S]
    in_maps = host_prep(inputs, S)
    trace = bool(os.environ.get("MLA_TRACE"))
    res = run_bass_kernel_spmd(
        nc, in_maps, core_ids=list(range(NCORES)), trace=trace
    )
    if trace:
        print(f"HW exec time: {res.exec_time_ns} ns")
        print(f"trace: {res.instructions_and_trace[1] if res.instructions_and_trace else None}")
    y = np.empty((B, S, D), np.float32)
    for b in range(B):
        acc = res.results[4 * b]["yT"].astype(np.float32).copy()
        for g in range(1, 4):
            acc += res.results[4 * b + g]["yT"].astype(np.float32)
        y[b] = acc.T
    return y
